# revision 35
# baseline (speedup 1.0000x reference)
"""Trainium2 Bass kernel for nn_Attention_4037269258732 (GQA attention with
RoPE, causal mask, and per-head sink-logit LSE renormalization).

Problem:  B=1, S=2048, DIM=2048, H=32 q-heads, KVH=8 kv-heads, HD=64.
          out = Wo @ attn(RoPE(Wq x), RoPE(Wk x), Wv x) + bo, causal,
          with out rows scaled by sigmoid(lse - sink_h).

Sharding (8 cores, tensor-parallel over heads):
  core c owns q-heads [4c, 4c+4), kv-head c, the matching rows of
  wq/wk/wv, wo's input-dim slice [256c, 256c+256), and sinks[4c:4c+4].
  Each core computes a full-shape [S, DIM] fp32 partial of the output
  projection; the host sums the 8 partials and adds wo_b once.

Device dataflow per core (feature dims on SBUF partitions; projection
work for seq block sb+1 is interleaved into the attention tile stream
of block sb as filler "work units" so the in-order PE never idles):
  Projection (per 512-col seq block sb):
    qT[256,S], kT[64,S], vT[64,S] = W.T @ xT   (PSUM accumulate over 16
    DIM chunks; bias folded into the ScalarE Identity eviction)
    RoPE via PE: rot_half(q) = Pblk.T @ q_raw (signed permutation as
    stationary), then qp = q_raw*cos + rot*sin (3 bf16 DVE multiplies)
    v transposed into Vext = [v_nat | 1] via PE transpose
  Attention (per block b, per head-pair pass p, per 128-row sk tile t):
    scores: two K=64 matmuls row-packed via tile_position into one
    [128,1024] PSUM pair; causal mask added in PSUM by an extra
    I.T @ (-1e4*tril) accumulate on diagonal tiles
    ptt = exp(scores/8) as ONE [128,~1024] ACTIVATE
    pso[h] += Vext_t.T @ ptt   (row 64 = sum_exp denominator; emitted
    depth-2 software-pipelined: av(t-2) issues before sc(t))
    per pass: denom rows (+e^sink) -> Ln -> Exp(-x); the K=1 broadcast
    matmul + outstk = pso * rinv (bf16) is deferred into the next pass
    wo(b) woven into block b+1's unit stream:
    psf[sq,dim] = outstk0.T@wo0 + outstk1.T@wo1, evicted fp32 -> DRAM
    (wo_b added host-side, free)
"""

import numpy as np
import ml_dtypes

import bass_rust
import concourse.bass as bass
import concourse.tile as tile
from concourse import mybir
from concourse.bass_utils import run_bass_kernel_spmd

F32 = mybir.dt.float32
BF16 = mybir.dt.bfloat16
AF = mybir.ActivationFunctionType
OP = mybir.AluOpType
BF = ml_dtypes.bfloat16

B, S, DIM = 1, 2048, 2048
H, KVH, HD = 32, 8, 64
NCORES = 8
QH = H // NCORES          # 4 q heads per core
SBLK = 512                # sq block size
NSB = S // SBLK           # 4
NDC = DIM // 128          # 16 contraction chunks
SCALE = 1.0 / 8.0         # 1/sqrt(HD)
MASKNEG = -10000.0

_ws_ctr = [0]


def _fix_range_clears(nc):
    """walrus here rejects the EVENT_SEMAPHORE_RANGE_CLEAR ISA struct
    ("ISA wrong length"); replace with per-sem write-0 NoOps."""
    import re as _re
    for f in nc.m.functions:
        for blk in f.blocks:
            out, changed = [], False
            for inst in blk.instructions:
                if type(inst).__name__ == "InstISA" and inst.isa_opcode == 176:
                    m = _re.search(r"range_first=(\d+) range_last=(\d+)", inst.concise())
                    first, last = int(m.group(1)), int(m.group(2))
                    for semid in range(first, last + 1):
                        _ws_ctr[0] += 1
                        nop = mybir.InstNoOp(name=f"I-rc-{_ws_ctr[0]}", ins=[], outs=[])
                        nop.engine = inst.engine
                        nop.sync_info = bass_rust.SyncInfo(
                            on_wait=[],
                            on_update=[
                                bass_rust.SyncUpdate(
                                    sync_type="semaphore",
                                    id=semid,
                                    update_mode="sem-wr-imm",
                                    update_value=0,
                                )
                            ],
                        )
                        out.append(nop)
                    changed = True
                    continue
                out.append(inst)
            if changed:
                blk.instructions = out


def _split_excess_waits(nc, max_waits=1):
    """walrus on this image encodes at most one SyncWait per instruction;
    hoist excess waits onto same-engine NoOps placed just before."""
    for f in nc.m.functions:
        for blk in f.blocks:
            out, changed = [], False
            for inst in blk.instructions:
                si = inst.sync_info
                waits = list(si.on_wait) if si is not None else []
                if len(waits) > max_waits:
                    excess, keep = waits[:-max_waits], waits[-max_waits:]
                    for k in range(0, len(excess), max_waits):
                        _ws_ctr[0] += 1
                        nop = mybir.InstNoOp(name=f"I-ws-{_ws_ctr[0]}", ins=[], outs=[])
                        nop.engine = inst.engine
                        nop.sync_info = bass_rust.SyncInfo(
                            on_wait=excess[k : k + max_waits], on_update=[]
                        )
                        out.append(nop)
                    inst.sync_info = bass_rust.SyncInfo(
                        on_wait=keep, on_update=list(si.on_update)
                    )
                    changed = True
                out.append(inst)
            if changed:
                blk.instructions = out


def _rot_perm(nheads):
    """Signed permutation P with (P.T @ q)[d] = rot_half(q)[d] per head."""
    n = nheads * HD
    P = np.zeros((n, n), np.float32)
    for d in range(n):
        j, dh = d // HD, d % HD
        src = j * HD + (dh + 32) % HD
        P[src, d] = -1.0 if dh < 32 else 1.0
    return P


def prep_inputs(inputs):
    """Host-side sharding/layout prep. Returns per-core input maps."""
    x = np.asarray(inputs["x"], np.float32)
    rope = np.asarray(inputs["rope_cache"], np.float32)
    wq = np.asarray(inputs["wq_w"], np.float32)
    bq = np.asarray(inputs["wq_b"], np.float32)
    wk = np.asarray(inputs["wk_w"], np.float32)
    bk = np.asarray(inputs["wk_b"], np.float32)
    wv = np.asarray(inputs["wv_w"], np.float32)
    bv = np.asarray(inputs["wv_b"], np.float32)
    wo = np.asarray(inputs["wo_w"], np.float32)
    sinks = np.asarray(inputs["sinks"], np.float32)

    xT = np.ascontiguousarray(x[0].T).astype(BF)            # [DIM, S]
    cosT = rope[:, :HD].T                                   # [64, S]
    sinT = rope[:, HD:].T
    cos2 = np.ascontiguousarray(np.concatenate([cosT, cosT], 0)).astype(BF)
    sin2 = np.ascontiguousarray(np.concatenate([sinT, sinT], 0)).astype(BF)
    pblk = np.ascontiguousarray(_rot_perm(2)).astype(BF)    # [128,128]
    mneg = (MASKNEG * np.tril(np.ones((128, 128), np.float32), -1)).astype(BF)
    ident = np.eye(128, dtype=BF)

    in_maps = []
    for c in range(NCORES):
        qs = slice(c * QH * HD, (c + 1) * QH * HD)          # 256 q rows
        ks = slice(c * HD, (c + 1) * HD)                    # 64 kv rows
        # wproj columns: [q 256 | k 64 | v 64] = 384
        wproj = np.concatenate([wq[qs].T, wk[ks].T, wv[ks].T], axis=1)
        bcol = np.zeros((128, 3), np.float32)
        bcol[:, 0] = bq[qs][0:128]
        bcol[:, 1] = bq[qs][128:256]
        bcol[0:64, 2] = bk[ks]
        bcol[64:128, 2] = bv[ks]
        woT = np.ascontiguousarray(wo[:, qs].T).astype(BF)  # [256, DIM]
        esink = np.tile(np.exp(sinks[c * QH : (c + 1) * QH]).reshape(1, QH),
                        (128, 1))
        in_maps.append(
            {
                "xT": xT,
                "wproj": np.ascontiguousarray(wproj).astype(BF),
                "bproj": bcol,
                "cos2": cos2,
                "sin2": sin2,
                "pblk": pblk,
                "woT": woT,
                "esink": esink.astype(np.float32),
                "mneg": mneg,
                "idb": ident,
                "ones_f": np.ones((128, 128), np.float32),
                "onesb": np.ones((128, 1), BF),
            }
        )
    return in_maps


def build_nc(split_waits=True):
    nc = bass.Bass("TRN2", target_bir_lowering=False, debug=False, num_devices=NCORES)
    xT = nc.dram_tensor("xT", [DIM, S], BF16, kind="ExternalInput").ap()
    wproj = nc.dram_tensor("wproj", [DIM, 384], BF16, kind="ExternalInput").ap()
    bproj = nc.dram_tensor("bproj", [128, 3], F32, kind="ExternalInput").ap()
    cos2 = nc.dram_tensor("cos2", [128, S], BF16, kind="ExternalInput").ap()
    sin2 = nc.dram_tensor("sin2", [128, S], BF16, kind="ExternalInput").ap()
    pblk = nc.dram_tensor("pblk", [128, 128], BF16, kind="ExternalInput").ap()
    woT = nc.dram_tensor("woT", [2 * 128, DIM], BF16, kind="ExternalInput").ap()
    esink = nc.dram_tensor("esink", [128, QH], F32, kind="ExternalInput").ap()
    mneg = nc.dram_tensor("mneg", [128, 128], BF16, kind="ExternalInput").ap()
    idb = nc.dram_tensor("idb", [128, 128], BF16, kind="ExternalInput").ap()
    ones_f = nc.dram_tensor("ones_f", [128, 128], F32, kind="ExternalInput").ap()
    onesb = nc.dram_tensor("onesb", [128, 1], BF16, kind="ExternalInput").ap()
    out = nc.dram_tensor("out", [S, DIM], F32, kind="ExternalOutput").ap()

    NT = S // 128  # 16 sk tiles

    with tile.TileContext(nc) as tc:
        with (
            tc.tile_pool(name="persist", bufs=1) as P,
            tc.tile_pool(name="projw", bufs=1) as PW,
            tc.tile_pool(name="tmp", bufs=3) as TMP,
            tc.tile_pool(name="ptp", bufs=4) as PTP,
            tc.tile_pool(name="rows", bufs=2) as RP,
            tc.tile_pool(name="rbp", bufs=2) as RBP,
            tc.tile_pool(name="osp", bufs=2) as OS,
            tc.tile_pool(name="oev", bufs=4) as OE,
            # PSUM budget (8 banks): proj accum 2 + scores [128,1024]=2
            # + pso 2 + mix (rot/vtrans/rb/psf) 2
            tc.tile_pool(name="pp", bufs=2, space="PSUM") as PP,
            tc.tile_pool(name="pss", bufs=1, space="PSUM") as PSS,
            tc.tile_pool(name="pso", bufs=1, space="PSUM") as PSO,
            tc.tile_pool(name="mix", bufs=2, space="PSUM") as MIX,
        ):
            esink_t = P.tile([128, QH], F32, tag="esink")
            mneg_t = P.tile([128, 128], BF16, tag="mneg")
            idb_t = P.tile([128, 128], BF16, tag="idb")
            wo_t = [P.tile([128, DIM], BF16, name=f"wo{i}", tag=f"wo{i}")
                    for i in range(2)]
            ones_ft = P.tile([128, 128], F32, tag="ones_ft")
            cos_t = P.tile([128, S], BF16, tag="cos")
            sin_t = P.tile([128, S], BF16, tag="sin")
            pblk_t = P.tile([128, 128], BF16, tag="pblk")
            onesb_t = P.tile([128, 1], BF16, tag="onesb")
            scr = P.tile([1, 16], F32, tag="scr")
            qp = [P.tile([128, S], BF16, name=f"qp{i}", tag=f"qp{i}") for i in range(2)]
            kvraw = P.tile([128, S], BF16, tag="kvraw")
            kT2 = P.tile([128, S], BF16, tag="kT2")
            vext = P.tile([128, NT * (HD + 1)], BF16, tag="vext")

            # warm the natural_log_exp table set at t=0 (memset input so
            # it gates on nothing; the ~2.7us table load then finishes
            # before the first projection eviction needs ScalarE)
            nc.gpsimd.memset(scr[:], 1.0)
            nc.scalar.activation(scr[0:1, 0:3], scr[0:1, 0:3], AF.Exp)
            nc.scalar.activation(scr[0:1, 0:3], scr[0:1, 0:3], AF.Ln)
            nc.gpsimd.dma_start(esink_t[:], esink[:])
            nc.gpsimd.dma_start(mneg_t[:], mneg[:])
            nc.gpsimd.dma_start(idb_t[:], idb[:])
            nc.gpsimd.dma_start(ones_ft[:], ones_f[:])
            nc.gpsimd.dma_start(cos_t[:], cos2[:])
            nc.gpsimd.dma_start(sin_t[:], sin2[:])
            nc.gpsimd.dma_start(pblk_t[:], pblk[:])
            nc.gpsimd.dma_start(onesb_t[:], onesb[:])
            # ones columns of Vext (persist; v copies never touch them)
            for t in range(NT):
                nc.vector.tensor_copy(vext[:, t * 65 + 64 : t * 65 + 65], onesb_t[:])

            w_t, x_t = [], []
            for dc in range(NDC):
                wt = PW.tile([128, 384], BF16, name=f"w{dc}", tag=f"w{dc}")
                nc.gpsimd.dma_start(wt[:], wproj[dc * 128 : (dc + 1) * 128, :])
                w_t.append(wt)
            bcol_t = PW.tile([128, 3], F32, tag="bcol")
            nc.gpsimd.dma_start(bcol_t[:], bproj[:])
            for dc in range(NDC):
                xt = PW.tile([128, S], BF16, name=f"x{dc}", tag=f"x{dc}")
                nc.sync.dma_start(
                    xt[:, 0:SBLK], xT[dc * 128 : (dc + 1) * 128, 0:SBLK]
                )
                x_t.append(xt)
            for dc in range(NDC):
                nc.sync.dma_start(
                    x_t[dc][:, SBLK:S], xT[dc * 128 : (dc + 1) * 128, SBLK:S]
                )
            for i in range(2):
                nc.gpsimd.dma_start(wo_t[i][:], woT[i * 128 : (i + 1) * 128, :])

            # ---- projection work units (interleaved into attention) ----
            # unit kinds: "mm" = free filler; "act" = ScalarE eviction
            # (stop draining after it so its dependent PE op lands in a
            # later iteration); "pedep" = PE op depending on a recent
            # eviction (stop draining after it).
            def gen_proj_units(sb):
                ss = slice(sb * SBLK, (sb + 1) * SBLK)
                units = []
                acc = {}

                def chunk(j, c0, c1, dc):
                    if dc == 0:
                        acc[j] = PP.tile([128, SBLK], F32,
                                         name=f"pp{sb}_{j}", tag="pp")
                    nc.tensor.matmul(
                        acc[j][:], w_t[dc][:, c0:c1], x_t[dc][:, ss],
                        start=(dc == 0), stop=(dc == NDC - 1),
                    )

                def evict_q(i):
                    qr = TMP.tile([128, SBLK], BF16, name="qr", tag=f"qr{i}")
                    nc.scalar.activation(
                        qr[:], acc[i][:], AF.Identity, bias=bcol_t[:, i : i + 1]
                    )
                    acc[f"qr{i}"] = qr

                def rope_q(i):
                    qr = acc[f"qr{i}"]
                    psr = MIX.tile([128, SBLK], F32, name="psr", tag="mix")
                    nc.tensor.matmul(psr[:], pblk_t[:], qr[:],
                                     start=True, stop=True)
                    t2 = TMP.tile([128, SBLK], BF16, name="t2", tag="t2")
                    nc.vector.tensor_tensor(t2[:], psr[:], sin_t[:, ss], op=OP.mult)
                    t1 = TMP.tile([128, SBLK], BF16, name="t1", tag="t1")
                    nc.vector.tensor_tensor(t1[:], qr[:], cos_t[:, ss], op=OP.mult)
                    nc.vector.tensor_tensor(qp[i][:, ss], t1[:], t2[:], op=OP.add)

                def evict_kv():
                    nc.scalar.activation(
                        kvraw[:, ss], acc[2][:], AF.Identity, bias=bcol_t[:, 2:3]
                    )

                def rope_k():
                    psrk = MIX.tile([128, SBLK], F32, name="psrk", tag="mix")
                    nc.tensor.matmul(psrk[0:64, :], pblk_t[0:64, 0:64],
                                     kvraw[0:64, ss], start=True, stop=True)
                    t2k = TMP.tile([64, SBLK], BF16, name="t2k", tag="t2k")
                    nc.vector.tensor_tensor(
                        t2k[:], psrk[0:64, :], sin_t[0:64, ss], op=OP.mult
                    )
                    t1k = TMP.tile([64, SBLK], BF16, name="t1k", tag="t1k")
                    nc.vector.tensor_tensor(
                        t1k[:], kvraw[0:64, ss], cos_t[0:64, ss], op=OP.mult
                    )
                    nc.vector.tensor_tensor(kT2[0:64, ss], t1k[:], t2k[:], op=OP.add)
                    nc.vector.tensor_copy(kT2[64:128, ss], kT2[0:64, ss])

                def vtrans(t):
                    pv = MIX.tile([128, SBLK], F32, name="pv", tag="mix")
                    pvb = pv[:].bitcast(BF16)
                    nc.tensor.matmul(
                        pvb[:, 0:HD],
                        kvraw[64:128, t * 128 : (t + 1) * 128],
                        idb_t[64:128, 64:128],
                        is_transpose=True,
                        tile_position=(64, 0),
                    )
                    nc.vector.tensor_copy(
                        vext[:, t * 65 : t * 65 + 64], pvb[:, 0:HD]
                    )

                # self-spacing order: each eviction ("act") is followed by
                # a few matmul chunks of the NEXT output before the PE op
                # that depends on it, so the in-order PE never waits.
                for dc in range(NDC):
                    units.append(("mm", lambda dc=dc: chunk(0, 0, 128, dc)))
                units.append(("act", lambda: evict_q(0)))
                for dc in range(3):
                    units.append(("mm", lambda dc=dc: chunk(1, 128, 256, dc)))
                units.append(("pedep", lambda: rope_q(0)))
                for dc in range(3, NDC):
                    units.append(("mm", lambda dc=dc: chunk(1, 128, 256, dc)))
                units.append(("act", lambda: evict_q(1)))
                for dc in range(3):
                    units.append(("mm", lambda dc=dc: chunk(2, 256, 384, dc)))
                units.append(("pedep", lambda: rope_q(1)))
                for dc in range(3, NDC):
                    units.append(("mm", lambda dc=dc: chunk(2, 256, 384, dc)))
                units.append(("act", evict_kv))
                units.append(("pedep", rope_k))
                for t in range(4 * sb, 4 * sb + 4):
                    units.append(("pedep", lambda t=t: vtrans(t)))
                return units

            def emit_renorm2(job):
                """broadcast rinv rows + apply to pso -> outstk (bf16)"""
                pso_j, rowb, osk_p = job
                for j in range(2):
                    ps_rb = MIX.tile([128, SBLK], F32, name="ps_rb", tag="mix")
                    nc.tensor.matmul(
                        ps_rb[0:64, :],
                        ones_ft[64 * j : 64 * j + 1, 0:64],
                        rowb[64 * j : 64 * j + 1, :],
                        start=True, stop=True,
                        tile_position=(64 * j, 0),
                    )
                    rb = RBP.tile([64, SBLK], F32, name="rb", tag="rb")
                    nc.vector.tensor_copy(rb[:], ps_rb[0:64, :])
                    nc.vector.tensor_tensor(
                        osk_p[64 * j : 64 * j + 64, :],
                        pso_j[j][0:64, :],
                        rb[:],
                        op=OP.mult,
                    )

            def gen_wo_units(bb, osk):
                units = []

                def wo_st(sti):
                    st = 4 * bb + sti
                    stl = slice(sti * 128, sti * 128 + 128)
                    for dbp in range(2):
                        psf = [
                            MIX.tile([128, SBLK], F32, name="psf", tag="mix")
                            for _ in range(2)
                        ]
                        for half in range(2):
                            for k in range(2):
                                db = 2 * dbp + k
                                ds = slice(db * SBLK, (db + 1) * SBLK)
                                nc.tensor.matmul(
                                    psf[k][:],
                                    osk[half][:, stl],
                                    wo_t[half][:, ds],
                                    start=(half == 0),
                                    stop=(half == 1),
                                )
                        for k in range(2):
                            db = 2 * dbp + k
                            ds = slice(db * SBLK, (db + 1) * SBLK)
                            ot = OE.tile([128, SBLK], F32, name="ot", tag="oe")
                            nc.vector.tensor_copy(ot[:], psf[k][:])
                            nc.sync.dma_start(
                                out[st * 128 : (st + 1) * 128, ds], ot[:]
                            )

                for sti in range(4):
                    units.append(("mm", lambda sti=sti: wo_st(sti)))
                return units

            # ---- bootstrap: proj(0) fully, then the attention loop ----
            for kind, fn in gen_proj_units(0):
                fn()

            unit_q = []
            renorm_job = None
            outstk_prev = None
            for b in range(NSB):
                nt = 4 * b + 4
                bs = b * SBLK
                # everything queued for this block must be in before its
                # first scores (qp/kT2/vext of block b, wo of b-2)
                for kind, fn in unit_q:
                    fn()
                unit_q = []
                if b + 1 < NSB:
                    unit_q += gen_proj_units(b + 1)
                if outstk_prev is not None:
                    # weave wo(b-1) into the stream now (its input outstk
                    # is finalized by emit_renorm2 at t==0 below); spacing
                    # the wo tiles among proj units keeps fillers flowing
                    wou = gen_wo_units(b - 1, outstk_prev)
                    nq = []
                    while unit_q or wou:
                        take = 3
                        while unit_q and take > 0:
                            nq.append(unit_q.pop(0))
                            take -= 1
                        if wou:
                            nq.append(wou.pop(0))
                    unit_q = nq
                    outstk_prev = None
                iters_left = [2 * nt]
                osk = [
                    OS.tile([128, SBLK], BF16, name=f"os{p}", tag=f"os{p}")
                    for p in range(2)
                ]
                for p in range(2):
                    pso = [
                        PSO.tile([65, SBLK], F32, name=f"oo{j}", tag=f"oo{j}")
                        for j in range(2)
                    ]

                    def emit_av(tt, ooff, pt):
                        for lane in range(2):
                            lo = ooff if lane == 0 else SBLK
                            nc.tensor.matmul(
                                pso[lane][:, ooff:SBLK],
                                vext[:, tt * 65 : (tt + 1) * 65],
                                pt[:, lo : lo + SBLK - ooff],
                                start=(tt == 0),
                                stop=(tt == nt - 1),
                            )

                    pend = []
                    for t in range(nt):
                        off = 128 * (t - 4 * b) if t >= 4 * b else 0
                        diag = t >= 4 * b
                        tc0 = slice(t * 128, (t + 1) * 128)
                        # av first so a stalled sc never blocks it
                        if len(pend) == 2:
                            emit_av(*pend.pop(0))
                        # deferred renorm part 2 must precede this pass's
                        # first av (pso ring reuse ordering)
                        if renorm_job is not None and t == 0:
                            emit_renorm2(renorm_job)
                            renorm_job = None
                        # drain filler units (order is self-spacing); pace
                        # so the queue lasts the whole block, keeping the
                        # PE fed through the late exp-bound iterations
                        cap = -(-len(unit_q) // max(iters_left[0], 1))
                        cap = min(max(cap, 2), 8)
                        iters_left[0] -= 1
                        nmm = 0
                        while unit_q and nmm < cap:
                            kind, fn = unit_q.pop(0)
                            fn()
                            nmm += 1
                            if kind == "pedep":
                                break
                        pss = PSS.tile([128, 2 * SBLK], F32, name="pss", tag="ss")
                        for lane in range(2):
                            # lane 1 packed left so [off : 2*SBLK-off] is
                            # one contiguous valid region for the exp
                            lo = off if lane == 0 else SBLK
                            nc.tensor.matmul(
                                pss[:, lo : lo + SBLK - off],
                                kT2[64 * lane : 64 * lane + 64, tc0],
                                qp[p][64 * lane : 64 * lane + 64,
                                     bs + off : bs + SBLK],
                                start=True,
                                stop=not diag,
                                tile_position=(64 * lane, 0),
                            )
                            if diag:
                                nc.tensor.matmul(
                                    pss[:, lo : lo + 128],
                                    idb_t[:],
                                    mneg_t[:],
                                    start=False,
                                    stop=True,
                                )
                        ptt = PTP.tile([128, 2 * SBLK], BF16, name="ptt", tag="pt")
                        nc.scalar.activation(
                            ptt[:, off : 2 * SBLK - off],
                            pss[:, off : 2 * SBLK - off],
                            AF.Exp,
                            scale=SCALE,
                        )
                        pend.append((t, off, ptt))
                    for pe_ in pend:
                        emit_av(*pe_)
                    # ---- renorm part 1 for heads (2p, 2p+1) ----
                    rowb = RP.tile([128, SBLK], F32, name="rowb", tag="rowb")
                    nc.gpsimd.memset(rowb[:], 1.0)
                    for j in range(2):
                        nc.vector.tensor_scalar_add(
                            rowb[64 * j : 64 * j + 1, :],
                            pso[j][64:65, :],
                            esink_t[64:65, 2 * p + j : 2 * p + j + 1],
                        )
                    rln = RP.tile([128, SBLK], F32, name="rln", tag="rln")
                    nc.scalar.activation(rln[:], rowb[:], AF.Ln)
                    nc.scalar.activation(rowb[:], rln[:], AF.Exp, scale=-1.0)
                    renorm_job = (pso, rowb, osk[p])
                    if p == 0 and outstk_prev is not None:
                        # weave wo tiles into the remaining units as spacers
                        wou = gen_wo_units(b - 1, outstk_prev)
                        nq = []
                        while unit_q or wou:
                            if wou:
                                nq.append(wou.pop(0))
                            take = 2
                            while unit_q and take > 0:
                                u = unit_q.pop(0)
                                nq.append(u)
                                take -= 1
                        unit_q = nq
                        outstk_prev = None
                outstk_prev = osk
            # tail
            for kind, fn in unit_q:
                fn()
            emit_renorm2(renorm_job)
            for kind, fn in gen_wo_units(NSB - 1, outstk_prev):
                fn()

    _fix_range_clears(nc)
    if split_waits:
        _split_excess_waits(nc)
    return nc


_nc_cache = [None]


def kernel(**inputs):
    in_maps = prep_inputs(inputs)
    if _nc_cache[0] is None:
        _nc_cache[0] = build_nc()
    nc = _nc_cache[0]
    res = run_bass_kernel_spmd(nc, in_maps, list(range(NCORES)))
    acc = res.results[0]["out"].astype(np.float32)
    for i in range(1, NCORES):
        acc = acc + res.results[i]["out"]
    acc = acc + np.asarray(inputs["wo_b"], np.float32).reshape(1, DIM)
    return acc.reshape(B, S, DIM)


# revision 36
# speedup vs baseline: 1.0034x; 1.0034x over previous
"""Trainium2 Bass kernel for nn_Attention_4037269258732 (GQA attention with
RoPE, causal mask, and per-head sink-logit LSE renormalization).

Problem:  B=1, S=2048, DIM=2048, H=32 q-heads, KVH=8 kv-heads, HD=64.
          out = Wo @ attn(RoPE(Wq x), RoPE(Wk x), Wv x) + bo, causal,
          with out rows scaled by sigmoid(lse - sink_h).

Sharding (8 cores, tensor-parallel over heads):
  core c owns q-heads [4c, 4c+4), kv-head c, the matching rows of
  wq/wk/wv, wo's input-dim slice [256c, 256c+256), and sinks[4c:4c+4].
  Each core computes a full-shape [S, DIM] fp32 partial of the output
  projection; the host sums the 8 partials and adds wo_b once.

Device dataflow per core (feature dims on SBUF partitions; projection
work for seq block sb+1 is interleaved into the attention tile stream
of block sb as filler "work units" so the in-order PE never idles):
  Projection (per 512-col seq block sb):
    qT[256,S], kT[64,S], vT[64,S] = W.T @ xT   (PSUM accumulate over 16
    DIM chunks; bias folded into the ScalarE Identity eviction)
    RoPE via PE: rot_half(q) = Pblk.T @ q_raw (signed permutation as
    stationary), then qp = q_raw*cos + rot*sin (3 bf16 DVE multiplies)
    v transposed into Vext = [v_nat | 1] via PE transpose
  Attention (per block b, per head-pair pass p, per 128-row sk tile t):
    scores: two K=64 matmuls row-packed via tile_position into one
    [128,1024] PSUM pair; causal mask added in PSUM by an extra
    I.T @ (-1e4*tril) accumulate on diagonal tiles
    ptt = exp(scores/8) as ONE [128,~1024] ACTIVATE
    pso[h] += Vext_t.T @ ptt   (row 64 = sum_exp denominator; emitted
    depth-2 software-pipelined: av(t-2) issues before sc(t))
    per pass: denom rows (+e^sink) -> Ln -> Exp(-x); the K=1 broadcast
    matmul + outstk = pso * rinv (bf16) is deferred into the next pass
    wo(b) woven into block b+1's unit stream:
    psf[sq,dim] = outstk0.T@wo0 + outstk1.T@wo1, evicted fp32 -> DRAM
    (wo_b added host-side, free)
"""

import numpy as np
import ml_dtypes

import bass_rust
import concourse.bass as bass
import concourse.tile as tile
from concourse import mybir
from concourse.bass_utils import run_bass_kernel_spmd

F32 = mybir.dt.float32
BF16 = mybir.dt.bfloat16
AF = mybir.ActivationFunctionType
OP = mybir.AluOpType
BF = ml_dtypes.bfloat16

B, S, DIM = 1, 2048, 2048
H, KVH, HD = 32, 8, 64
NCORES = 8
QH = H // NCORES          # 4 q heads per core
SBLK = 512                # sq block size
NSB = S // SBLK           # 4
NDC = DIM // 128          # 16 contraction chunks
SCALE = 1.0 / 8.0         # 1/sqrt(HD)
MASKNEG = -10000.0

_ws_ctr = [0]


def _fix_range_clears(nc):
    """walrus here rejects the EVENT_SEMAPHORE_RANGE_CLEAR ISA struct
    ("ISA wrong length"); replace with per-sem write-0 NoOps."""
    import re as _re
    for f in nc.m.functions:
        for blk in f.blocks:
            out, changed = [], False
            for inst in blk.instructions:
                if type(inst).__name__ == "InstISA" and inst.isa_opcode == 176:
                    m = _re.search(r"range_first=(\d+) range_last=(\d+)", inst.concise())
                    first, last = int(m.group(1)), int(m.group(2))
                    for semid in range(first, last + 1):
                        _ws_ctr[0] += 1
                        nop = mybir.InstNoOp(name=f"I-rc-{_ws_ctr[0]}", ins=[], outs=[])
                        nop.engine = inst.engine
                        nop.sync_info = bass_rust.SyncInfo(
                            on_wait=[],
                            on_update=[
                                bass_rust.SyncUpdate(
                                    sync_type="semaphore",
                                    id=semid,
                                    update_mode="sem-wr-imm",
                                    update_value=0,
                                )
                            ],
                        )
                        out.append(nop)
                    changed = True
                    continue
                out.append(inst)
            if changed:
                blk.instructions = out


def _split_excess_waits(nc, max_waits=1):
    """walrus on this image encodes at most one SyncWait per instruction;
    hoist excess waits onto same-engine NoOps placed just before."""
    for f in nc.m.functions:
        for blk in f.blocks:
            out, changed = [], False
            for inst in blk.instructions:
                si = inst.sync_info
                waits = list(si.on_wait) if si is not None else []
                if len(waits) > max_waits:
                    excess, keep = waits[:-max_waits], waits[-max_waits:]
                    for k in range(0, len(excess), max_waits):
                        _ws_ctr[0] += 1
                        nop = mybir.InstNoOp(name=f"I-ws-{_ws_ctr[0]}", ins=[], outs=[])
                        nop.engine = inst.engine
                        nop.sync_info = bass_rust.SyncInfo(
                            on_wait=excess[k : k + max_waits], on_update=[]
                        )
                        out.append(nop)
                    inst.sync_info = bass_rust.SyncInfo(
                        on_wait=keep, on_update=list(si.on_update)
                    )
                    changed = True
                out.append(inst)
            if changed:
                blk.instructions = out


def _rot_perm(nheads):
    """Signed permutation P with (P.T @ q)[d] = rot_half(q)[d] per head."""
    n = nheads * HD
    P = np.zeros((n, n), np.float32)
    for d in range(n):
        j, dh = d // HD, d % HD
        src = j * HD + (dh + 32) % HD
        P[src, d] = -1.0 if dh < 32 else 1.0
    return P


def prep_inputs(inputs):
    """Host-side sharding/layout prep. Returns per-core input maps."""
    x = np.asarray(inputs["x"], np.float32)
    rope = np.asarray(inputs["rope_cache"], np.float32)
    wq = np.asarray(inputs["wq_w"], np.float32)
    bq = np.asarray(inputs["wq_b"], np.float32)
    wk = np.asarray(inputs["wk_w"], np.float32)
    bk = np.asarray(inputs["wk_b"], np.float32)
    wv = np.asarray(inputs["wv_w"], np.float32)
    bv = np.asarray(inputs["wv_b"], np.float32)
    wo = np.asarray(inputs["wo_w"], np.float32)
    sinks = np.asarray(inputs["sinks"], np.float32)

    xT = np.ascontiguousarray(x[0].T).astype(BF)            # [DIM, S]
    cosT = rope[:, :HD].T                                   # [64, S]
    sinT = rope[:, HD:].T
    cos2 = np.ascontiguousarray(np.concatenate([cosT, cosT], 0)).astype(BF)
    sin2 = np.ascontiguousarray(np.concatenate([sinT, sinT], 0)).astype(BF)
    pblk = np.ascontiguousarray(_rot_perm(2)).astype(BF)    # [128,128]
    mneg = (MASKNEG * np.tril(np.ones((128, 128), np.float32), -1)).astype(BF)
    ident = np.eye(128, dtype=BF)

    in_maps = []
    for c in range(NCORES):
        qs = slice(c * QH * HD, (c + 1) * QH * HD)          # 256 q rows
        ks = slice(c * HD, (c + 1) * HD)                    # 64 kv rows
        # wproj columns: [q 256 | k 64 | v 64] = 384
        wproj = np.concatenate([wq[qs].T, wk[ks].T, wv[ks].T], axis=1)
        bcol = np.zeros((128, 3), np.float32)
        bcol[:, 0] = bq[qs][0:128]
        bcol[:, 1] = bq[qs][128:256]
        bcol[0:64, 2] = bk[ks]
        bcol[64:128, 2] = bv[ks]
        woT = np.ascontiguousarray(wo[:, qs].T).astype(BF)  # [256, DIM]
        esink = np.tile(np.exp(sinks[c * QH : (c + 1) * QH]).reshape(1, QH),
                        (128, 1))
        in_maps.append(
            {
                "xT": xT,
                "wproj": np.ascontiguousarray(wproj).astype(BF),
                "bproj": bcol,
                "cos2": cos2,
                "sin2": sin2,
                "pblk": pblk,
                "woT": woT,
                "esink": esink.astype(np.float32),
                "mneg": mneg,
                "idb": ident,
                "ones_f": np.ones((128, 128), np.float32),
                "onesb": np.ones((128, 1), BF),
            }
        )
    return in_maps


def build_nc(split_waits=True):
    nc = bass.Bass("TRN2", target_bir_lowering=False, debug=False, num_devices=NCORES)
    xT = nc.dram_tensor("xT", [DIM, S], BF16, kind="ExternalInput").ap()
    wproj = nc.dram_tensor("wproj", [DIM, 384], BF16, kind="ExternalInput").ap()
    bproj = nc.dram_tensor("bproj", [128, 3], F32, kind="ExternalInput").ap()
    cos2 = nc.dram_tensor("cos2", [128, S], BF16, kind="ExternalInput").ap()
    sin2 = nc.dram_tensor("sin2", [128, S], BF16, kind="ExternalInput").ap()
    pblk = nc.dram_tensor("pblk", [128, 128], BF16, kind="ExternalInput").ap()
    woT = nc.dram_tensor("woT", [2 * 128, DIM], BF16, kind="ExternalInput").ap()
    esink = nc.dram_tensor("esink", [128, QH], F32, kind="ExternalInput").ap()
    mneg = nc.dram_tensor("mneg", [128, 128], BF16, kind="ExternalInput").ap()
    idb = nc.dram_tensor("idb", [128, 128], BF16, kind="ExternalInput").ap()
    ones_f = nc.dram_tensor("ones_f", [128, 128], F32, kind="ExternalInput").ap()
    onesb = nc.dram_tensor("onesb", [128, 1], BF16, kind="ExternalInput").ap()
    out = nc.dram_tensor("out", [S, DIM], F32, kind="ExternalOutput").ap()

    NT = S // 128  # 16 sk tiles

    with tile.TileContext(nc) as tc:
        with (
            tc.tile_pool(name="persist", bufs=1) as P,
            tc.tile_pool(name="projw", bufs=1) as PW,
            tc.tile_pool(name="tmp", bufs=3) as TMP,
            tc.tile_pool(name="ptp", bufs=4) as PTP,
            tc.tile_pool(name="rows", bufs=2) as RP,
            tc.tile_pool(name="rbp", bufs=2) as RBP,
            tc.tile_pool(name="osp", bufs=2) as OS,
            tc.tile_pool(name="oev", bufs=4) as OE,
            # PSUM budget (8 banks): proj accum 2 + scores [128,1024]=2
            # + pso 2 + mix (rot/vtrans/rb/psf) 2
            tc.tile_pool(name="pp", bufs=2, space="PSUM") as PP,
            tc.tile_pool(name="pss", bufs=1, space="PSUM") as PSS,
            tc.tile_pool(name="pso", bufs=1, space="PSUM") as PSO,
            tc.tile_pool(name="mix", bufs=2, space="PSUM") as MIX,
        ):
            esink_t = P.tile([128, QH], F32, tag="esink")
            mneg_t = P.tile([128, 128], BF16, tag="mneg")
            idb_t = P.tile([128, 128], BF16, tag="idb")
            wo_t = [P.tile([128, DIM], BF16, name=f"wo{i}", tag=f"wo{i}")
                    for i in range(2)]
            ones_ft = P.tile([128, 128], F32, tag="ones_ft")
            cos_t = P.tile([128, S], BF16, tag="cos")
            sin_t = P.tile([128, S], BF16, tag="sin")
            pblk_t = P.tile([128, 128], BF16, tag="pblk")
            onesb_t = P.tile([128, 1], BF16, tag="onesb")
            scr = P.tile([1, 16], F32, tag="scr")
            qp = [P.tile([128, S], BF16, name=f"qp{i}", tag=f"qp{i}") for i in range(2)]
            kvraw = P.tile([128, S], BF16, tag="kvraw")
            kT2 = P.tile([128, S], BF16, tag="kT2")
            vext = P.tile([128, NT * (HD + 1)], BF16, tag="vext")

            nc.gpsimd.dma_start(esink_t[:], esink[:])
            nc.gpsimd.dma_start(mneg_t[:], mneg[:])
            nc.gpsimd.dma_start(idb_t[:], idb[:])
            nc.gpsimd.dma_start(ones_ft[:], ones_f[:])
            nc.gpsimd.dma_start(cos_t[:], cos2[:])
            nc.gpsimd.dma_start(sin_t[:], sin2[:])
            nc.gpsimd.dma_start(pblk_t[:], pblk[:])
            nc.gpsimd.dma_start(onesb_t[:], onesb[:])
            # warm the natural_log_exp table set off the critical path
            nc.scalar.activation(scr[0:1, 0:3], esink_t[0:1, 0:3], AF.Exp)
            nc.scalar.activation(scr[0:1, 0:3], scr[0:1, 0:3], AF.Ln)
            # ones columns of Vext (persist; v copies never touch them)
            for t in range(NT):
                nc.vector.tensor_copy(vext[:, t * 65 + 64 : t * 65 + 65], onesb_t[:])

            w_t, x_t = [], []
            for dc in range(NDC):
                wt = PW.tile([128, 384], BF16, name=f"w{dc}", tag=f"w{dc}")
                nc.gpsimd.dma_start(wt[:], wproj[dc * 128 : (dc + 1) * 128, :])
                w_t.append(wt)
            bcol_t = PW.tile([128, 3], F32, tag="bcol")
            nc.gpsimd.dma_start(bcol_t[:], bproj[:])
            for dc in range(NDC):
                xt = PW.tile([128, S], BF16, name=f"x{dc}", tag=f"x{dc}")
                nc.sync.dma_start(
                    xt[:, 0:SBLK], xT[dc * 128 : (dc + 1) * 128, 0:SBLK]
                )
                x_t.append(xt)
            for dc in range(NDC):
                nc.sync.dma_start(
                    x_t[dc][:, SBLK:S], xT[dc * 128 : (dc + 1) * 128, SBLK:S]
                )
            for i in range(2):
                nc.gpsimd.dma_start(wo_t[i][:], woT[i * 128 : (i + 1) * 128, :])

            # ---- projection work units (interleaved into attention) ----
            # unit kinds: "mm" = free filler; "act" = ScalarE eviction
            # (stop draining after it so its dependent PE op lands in a
            # later iteration); "pedep" = PE op depending on a recent
            # eviction (stop draining after it).
            def gen_proj_units(sb):
                ss = slice(sb * SBLK, (sb + 1) * SBLK)
                units = []
                acc = {}

                def chunk(j, c0, c1, dc):
                    if dc == 0:
                        acc[j] = PP.tile([128, SBLK], F32,
                                         name=f"pp{sb}_{j}", tag="pp")
                    nc.tensor.matmul(
                        acc[j][:], w_t[dc][:, c0:c1], x_t[dc][:, ss],
                        start=(dc == 0), stop=(dc == NDC - 1),
                    )

                def evict_q(i):
                    qr = TMP.tile([128, SBLK], BF16, name="qr", tag=f"qr{i}")
                    nc.scalar.activation(
                        qr[:], acc[i][:], AF.Identity, bias=bcol_t[:, i : i + 1]
                    )
                    acc[f"qr{i}"] = qr

                def rope_q(i):
                    qr = acc[f"qr{i}"]
                    psr = MIX.tile([128, SBLK], F32, name="psr", tag="mix")
                    nc.tensor.matmul(psr[:], pblk_t[:], qr[:],
                                     start=True, stop=True)
                    t2 = TMP.tile([128, SBLK], BF16, name="t2", tag="t2")
                    nc.vector.tensor_tensor(t2[:], psr[:], sin_t[:, ss], op=OP.mult)
                    t1 = TMP.tile([128, SBLK], BF16, name="t1", tag="t1")
                    nc.vector.tensor_tensor(t1[:], qr[:], cos_t[:, ss], op=OP.mult)
                    nc.vector.tensor_tensor(qp[i][:, ss], t1[:], t2[:], op=OP.add)

                def evict_kv():
                    nc.scalar.activation(
                        kvraw[:, ss], acc[2][:], AF.Identity, bias=bcol_t[:, 2:3]
                    )

                def rope_k():
                    psrk = MIX.tile([128, SBLK], F32, name="psrk", tag="mix")
                    nc.tensor.matmul(psrk[0:64, :], pblk_t[0:64, 0:64],
                                     kvraw[0:64, ss], start=True, stop=True)
                    t2k = TMP.tile([64, SBLK], BF16, name="t2k", tag="t2k")
                    nc.vector.tensor_tensor(
                        t2k[:], psrk[0:64, :], sin_t[0:64, ss], op=OP.mult
                    )
                    t1k = TMP.tile([64, SBLK], BF16, name="t1k", tag="t1k")
                    nc.vector.tensor_tensor(
                        t1k[:], kvraw[0:64, ss], cos_t[0:64, ss], op=OP.mult
                    )
                    nc.vector.tensor_tensor(kT2[0:64, ss], t1k[:], t2k[:], op=OP.add)
                    nc.vector.tensor_copy(kT2[64:128, ss], kT2[0:64, ss])

                def vtrans(t):
                    pv = MIX.tile([128, SBLK], F32, name="pv", tag="mix")
                    pvb = pv[:].bitcast(BF16)
                    nc.tensor.matmul(
                        pvb[:, 0:HD],
                        kvraw[64:128, t * 128 : (t + 1) * 128],
                        idb_t[64:128, 64:128],
                        is_transpose=True,
                        tile_position=(64, 0),
                    )
                    nc.vector.tensor_copy(
                        vext[:, t * 65 : t * 65 + 64], pvb[:, 0:HD]
                    )

                # self-spacing order: each eviction ("act") is followed by
                # a few matmul chunks of the NEXT output before the PE op
                # that depends on it, so the in-order PE never waits.
                for dc in range(NDC):
                    units.append(("mm", lambda dc=dc: chunk(0, 0, 128, dc)))
                units.append(("act", lambda: evict_q(0)))
                for dc in range(3):
                    units.append(("mm", lambda dc=dc: chunk(1, 128, 256, dc)))
                units.append(("pedep", lambda: rope_q(0)))
                for dc in range(3, NDC):
                    units.append(("mm", lambda dc=dc: chunk(1, 128, 256, dc)))
                units.append(("act", lambda: evict_q(1)))
                for dc in range(3):
                    units.append(("mm", lambda dc=dc: chunk(2, 256, 384, dc)))
                units.append(("pedep", lambda: rope_q(1)))
                for dc in range(3, NDC):
                    units.append(("mm", lambda dc=dc: chunk(2, 256, 384, dc)))
                units.append(("act", evict_kv))
                units.append(("pedep", rope_k))
                for t in range(4 * sb, 4 * sb + 4):
                    units.append(("pedep", lambda t=t: vtrans(t)))
                return units

            def emit_renorm2(job):
                """broadcast rinv rows + apply to pso -> outstk (bf16)"""
                pso_j, rowb, osk_p = job
                for j in range(2):
                    ps_rb = MIX.tile([128, SBLK], F32, name="ps_rb", tag="mix")
                    nc.tensor.matmul(
                        ps_rb[0:64, :],
                        ones_ft[64 * j : 64 * j + 1, 0:64],
                        rowb[64 * j : 64 * j + 1, :],
                        start=True, stop=True,
                        tile_position=(64 * j, 0),
                    )
                    rb = RBP.tile([64, SBLK], F32, name="rb", tag="rb")
                    nc.vector.tensor_copy(rb[:], ps_rb[0:64, :])
                    nc.vector.tensor_tensor(
                        osk_p[64 * j : 64 * j + 64, :],
                        pso_j[j][0:64, :],
                        rb[:],
                        op=OP.mult,
                    )

            def gen_wo_units(bb, osk):
                units = []

                def wo_st(sti):
                    st = 4 * bb + sti
                    stl = slice(sti * 128, sti * 128 + 128)
                    for dbp in range(2):
                        psf = [
                            MIX.tile([128, SBLK], F32, name="psf", tag="mix")
                            for _ in range(2)
                        ]
                        for half in range(2):
                            for k in range(2):
                                db = 2 * dbp + k
                                ds = slice(db * SBLK, (db + 1) * SBLK)
                                nc.tensor.matmul(
                                    psf[k][:],
                                    osk[half][:, stl],
                                    wo_t[half][:, ds],
                                    start=(half == 0),
                                    stop=(half == 1),
                                )
                        for k in range(2):
                            db = 2 * dbp + k
                            ds = slice(db * SBLK, (db + 1) * SBLK)
                            ot = OE.tile([128, SBLK], F32, name="ot", tag="oe")
                            nc.vector.tensor_copy(ot[:], psf[k][:])
                            nc.sync.dma_start(
                                out[st * 128 : (st + 1) * 128, ds], ot[:]
                            )

                for sti in range(4):
                    units.append(("mm", lambda sti=sti: wo_st(sti)))
                return units

            # ---- bootstrap: proj(0) fully, then the attention loop ----
            for kind, fn in gen_proj_units(0):
                fn()

            unit_q = []
            renorm_job = None
            outstk_prev = None
            for b in range(NSB):
                nt = 4 * b + 4
                bs = b * SBLK
                # everything queued for this block must be in before its
                # first scores (qp/kT2/vext of block b, wo of b-2)
                for kind, fn in unit_q:
                    fn()
                unit_q = []
                if b + 1 < NSB:
                    unit_q += gen_proj_units(b + 1)
                if outstk_prev is not None:
                    # weave wo(b-1) into the stream now (its input outstk
                    # is finalized by emit_renorm2 at t==0 below); spacing
                    # the wo tiles among proj units keeps fillers flowing
                    wou = gen_wo_units(b - 1, outstk_prev)
                    nq = []
                    while unit_q or wou:
                        take = 3
                        while unit_q and take > 0:
                            nq.append(unit_q.pop(0))
                            take -= 1
                        if wou:
                            nq.append(wou.pop(0))
                    unit_q = nq
                    outstk_prev = None
                iters_left = [2 * nt]
                osk = [
                    OS.tile([128, SBLK], BF16, name=f"os{p}", tag=f"os{p}")
                    for p in range(2)
                ]
                for p in range(2):
                    pso = [
                        PSO.tile([65, SBLK], F32, name=f"oo{j}", tag=f"oo{j}")
                        for j in range(2)
                    ]

                    def emit_av(tt, ooff, pt):
                        for lane in range(2):
                            lo = ooff if lane == 0 else SBLK
                            nc.tensor.matmul(
                                pso[lane][:, ooff:SBLK],
                                vext[:, tt * 65 : (tt + 1) * 65],
                                pt[:, lo : lo + SBLK - ooff],
                                start=(tt == 0),
                                stop=(tt == nt - 1),
                            )

                    pend = []
                    for t in range(nt):
                        off = 128 * (t - 4 * b) if t >= 4 * b else 0
                        diag = t >= 4 * b
                        tc0 = slice(t * 128, (t + 1) * 128)
                        # av first so a stalled sc never blocks it
                        if len(pend) == 2:
                            emit_av(*pend.pop(0))
                        # deferred renorm part 2 must precede this pass's
                        # first av (pso ring reuse ordering)
                        if renorm_job is not None and t == 0:
                            emit_renorm2(renorm_job)
                            renorm_job = None
                        # drain filler units (order is self-spacing); pace
                        # so the queue lasts the whole block, keeping the
                        # PE fed through the late exp-bound iterations
                        cap = -(-len(unit_q) // max(iters_left[0], 1))
                        cap = min(max(cap, 2), 8)
                        iters_left[0] -= 1
                        nmm = 0
                        while unit_q and nmm < cap:
                            kind, fn = unit_q.pop(0)
                            fn()
                            nmm += 1
                            if kind == "pedep":
                                break
                        pss = PSS.tile([128, 2 * SBLK], F32, name="pss", tag="ss")
                        for lane in range(2):
                            # lane 1 packed left so [off : 2*SBLK-off] is
                            # one contiguous valid region for the exp
                            lo = off if lane == 0 else SBLK
                            nc.tensor.matmul(
                                pss[:, lo : lo + SBLK - off],
                                kT2[64 * lane : 64 * lane + 64, tc0],
                                qp[p][64 * lane : 64 * lane + 64,
                                     bs + off : bs + SBLK],
                                start=True,
                                stop=not diag,
                                tile_position=(64 * lane, 0),
                            )
                            if diag:
                                nc.tensor.matmul(
                                    pss[:, lo : lo + 128],
                                    idb_t[:],
                                    mneg_t[:],
                                    start=False,
                                    stop=True,
                                )
                        ptt = PTP.tile([128, 2 * SBLK], BF16, name="ptt", tag="pt")
                        nc.scalar.activation(
                            ptt[:, off : 2 * SBLK - off],
                            pss[:, off : 2 * SBLK - off],
                            AF.Exp,
                            scale=SCALE,
                        )
                        pend.append((t, off, ptt))
                    for pe_ in pend:
                        emit_av(*pe_)
                    # ---- renorm part 1 for heads (2p, 2p+1) ----
                    rowb = RP.tile([128, SBLK], F32, name="rowb", tag="rowb")
                    nc.gpsimd.memset(rowb[:], 1.0)
                    for j in range(2):
                        nc.vector.tensor_scalar_add(
                            rowb[64 * j : 64 * j + 1, :],
                            pso[j][64:65, :],
                            esink_t[64:65, 2 * p + j : 2 * p + j + 1],
                        )
                    rln = RP.tile([128, SBLK], F32, name="rln", tag="rln")
                    nc.scalar.activation(rln[:], rowb[:], AF.Ln)
                    nc.scalar.activation(rowb[:], rln[:], AF.Exp, scale=-1.0)
                    renorm_job = (pso, rowb, osk[p])
                    if p == 0 and outstk_prev is not None:
                        # weave wo tiles into the remaining units as spacers
                        wou = gen_wo_units(b - 1, outstk_prev)
                        nq = []
                        while unit_q or wou:
                            if wou:
                                nq.append(wou.pop(0))
                            take = 2
                            while unit_q and take > 0:
                                u = unit_q.pop(0)
                                nq.append(u)
                                take -= 1
                        unit_q = nq
                        outstk_prev = None
                outstk_prev = osk
            # tail
            for kind, fn in unit_q:
                fn()
            emit_renorm2(renorm_job)
            for kind, fn in gen_wo_units(NSB - 1, outstk_prev):
                fn()

    _fix_range_clears(nc)
    if split_waits:
        _split_excess_waits(nc)
    return nc


_nc_cache = [None]


def kernel(**inputs):
    in_maps = prep_inputs(inputs)
    if _nc_cache[0] is None:
        _nc_cache[0] = build_nc()
    nc = _nc_cache[0]
    res = run_bass_kernel_spmd(nc, in_maps, list(range(NCORES)))
    acc = res.results[0]["out"].astype(np.float32)
    for i in range(1, NCORES):
        acc = acc + res.results[i]["out"]
    acc = acc + np.asarray(inputs["wo_b"], np.float32).reshape(1, DIM)
    return acc.reshape(B, S, DIM)


# revision 38
# speedup vs baseline: 1.0066x; 1.0031x over previous
"""Trainium2 Bass kernel for nn_Attention_4037269258732 (GQA attention with
RoPE, causal mask, and per-head sink-logit LSE renormalization).

Problem:  B=1, S=2048, DIM=2048, H=32 q-heads, KVH=8 kv-heads, HD=64.
          out = Wo @ attn(RoPE(Wq x), RoPE(Wk x), Wv x) + bo, causal,
          with out rows scaled by sigmoid(lse - sink_h).

Sharding (8 cores, tensor-parallel over heads):
  core c owns q-heads [4c, 4c+4), kv-head c, the matching rows of
  wq/wk/wv, wo's input-dim slice [256c, 256c+256), and sinks[4c:4c+4].
  Each core computes a full-shape [S, DIM] fp32 partial of the output
  projection; the host sums the 8 partials and adds wo_b once.

Device dataflow per core (feature dims on SBUF partitions; projection
work for seq block sb+1 is interleaved into the attention tile stream
of block sb as filler "work units" so the in-order PE never idles):
  Projection (per 512-col seq block sb):
    qT[256,S], kT[64,S], vT[64,S] = W.T @ xT   (PSUM accumulate over 16
    DIM chunks; bias folded into the ScalarE Identity eviction)
    RoPE via PE: rot_half(q) = Pblk.T @ q_raw (signed permutation as
    stationary), then qp = q_raw*cos + rot*sin (3 bf16 DVE multiplies)
    v transposed into Vext = [v_nat | 1] via PE transpose
  Attention (per block b, per head-pair pass p, per 128-row sk tile t):
    scores: two K=64 matmuls row-packed via tile_position into one
    [128,1024] PSUM pair; causal mask added in PSUM by an extra
    I.T @ (-1e4*tril) accumulate on diagonal tiles
    ptt = exp(scores/8) as ONE [128,~1024] ACTIVATE
    pso[h] += Vext_t.T @ ptt   (row 64 = sum_exp denominator; emitted
    depth-2 software-pipelined: av(t-2) issues before sc(t))
    per pass: denom rows (+e^sink) -> Ln -> Exp(-x); the K=1 broadcast
    matmul + outstk = pso * rinv (bf16) is deferred into the next pass
    wo(b) woven into block b+1's unit stream:
    psf[sq,dim] = outstk0.T@wo0 + outstk1.T@wo1, evicted fp32 -> DRAM
    (wo_b added host-side, free)
"""

import numpy as np
import ml_dtypes

import bass_rust
import concourse.bass as bass
import concourse.tile as tile
from concourse import mybir
from concourse.bass_utils import run_bass_kernel_spmd

F32 = mybir.dt.float32
BF16 = mybir.dt.bfloat16
AF = mybir.ActivationFunctionType
OP = mybir.AluOpType
BF = ml_dtypes.bfloat16

B, S, DIM = 1, 2048, 2048
H, KVH, HD = 32, 8, 64
NCORES = 8
QH = H // NCORES          # 4 q heads per core
SBLK = 512                # sq block size
NSB = S // SBLK           # 4
NDC = DIM // 128          # 16 contraction chunks
SCALE = 1.0 / 8.0         # 1/sqrt(HD)
MASKNEG = -10000.0

_ws_ctr = [0]


def _fix_range_clears(nc):
    """walrus here rejects the EVENT_SEMAPHORE_RANGE_CLEAR ISA struct
    ("ISA wrong length"); replace with per-sem write-0 NoOps."""
    import re as _re
    for f in nc.m.functions:
        for blk in f.blocks:
            out, changed = [], False
            for inst in blk.instructions:
                if type(inst).__name__ == "InstISA" and inst.isa_opcode == 176:
                    m = _re.search(r"range_first=(\d+) range_last=(\d+)", inst.concise())
                    first, last = int(m.group(1)), int(m.group(2))
                    for semid in range(first, last + 1):
                        _ws_ctr[0] += 1
                        nop = mybir.InstNoOp(name=f"I-rc-{_ws_ctr[0]}", ins=[], outs=[])
                        nop.engine = inst.engine
                        nop.sync_info = bass_rust.SyncInfo(
                            on_wait=[],
                            on_update=[
                                bass_rust.SyncUpdate(
                                    sync_type="semaphore",
                                    id=semid,
                                    update_mode="sem-wr-imm",
                                    update_value=0,
                                )
                            ],
                        )
                        out.append(nop)
                    changed = True
                    continue
                out.append(inst)
            if changed:
                blk.instructions = out


def _split_excess_waits(nc, max_waits=1):
    """walrus on this image encodes at most one SyncWait per instruction;
    hoist excess waits onto same-engine NoOps placed just before."""
    for f in nc.m.functions:
        for blk in f.blocks:
            out, changed = [], False
            for inst in blk.instructions:
                si = inst.sync_info
                waits = list(si.on_wait) if si is not None else []
                if len(waits) > max_waits:
                    excess, keep = waits[:-max_waits], waits[-max_waits:]
                    for k in range(0, len(excess), max_waits):
                        _ws_ctr[0] += 1
                        nop = mybir.InstNoOp(name=f"I-ws-{_ws_ctr[0]}", ins=[], outs=[])
                        nop.engine = inst.engine
                        nop.sync_info = bass_rust.SyncInfo(
                            on_wait=excess[k : k + max_waits], on_update=[]
                        )
                        out.append(nop)
                    inst.sync_info = bass_rust.SyncInfo(
                        on_wait=keep, on_update=list(si.on_update)
                    )
                    changed = True
                out.append(inst)
            if changed:
                blk.instructions = out


def _rot_perm(nheads):
    """Signed permutation P with (P.T @ q)[d] = rot_half(q)[d] per head."""
    n = nheads * HD
    P = np.zeros((n, n), np.float32)
    for d in range(n):
        j, dh = d // HD, d % HD
        src = j * HD + (dh + 32) % HD
        P[src, d] = -1.0 if dh < 32 else 1.0
    return P


def prep_inputs(inputs):
    """Host-side sharding/layout prep. Returns per-core input maps."""
    x = np.asarray(inputs["x"], np.float32)
    rope = np.asarray(inputs["rope_cache"], np.float32)
    wq = np.asarray(inputs["wq_w"], np.float32)
    bq = np.asarray(inputs["wq_b"], np.float32)
    wk = np.asarray(inputs["wk_w"], np.float32)
    bk = np.asarray(inputs["wk_b"], np.float32)
    wv = np.asarray(inputs["wv_w"], np.float32)
    bv = np.asarray(inputs["wv_b"], np.float32)
    wo = np.asarray(inputs["wo_w"], np.float32)
    sinks = np.asarray(inputs["sinks"], np.float32)

    xT = np.ascontiguousarray(x[0].T).astype(BF)            # [DIM, S]
    cosT = rope[:, :HD].T                                   # [64, S]
    sinT = rope[:, HD:].T
    cos2 = np.ascontiguousarray(np.concatenate([cosT, cosT], 0)).astype(BF)
    sin2 = np.ascontiguousarray(np.concatenate([sinT, sinT], 0)).astype(BF)
    pblk = np.ascontiguousarray(_rot_perm(2)).astype(BF)    # [128,128]
    mneg = (MASKNEG * np.tril(np.ones((128, 128), np.float32), -1)).astype(BF)
    ident = np.eye(128, dtype=BF)

    in_maps = []
    for c in range(NCORES):
        qs = slice(c * QH * HD, (c + 1) * QH * HD)          # 256 q rows
        ks = slice(c * HD, (c + 1) * HD)                    # 64 kv rows
        # wproj columns: [q 256 | k 64 | v 64] = 384
        wproj = np.concatenate([wq[qs].T, wk[ks].T, wv[ks].T], axis=1)
        bcol = np.zeros((128, 3), np.float32)
        bcol[:, 0] = bq[qs][0:128]
        bcol[:, 1] = bq[qs][128:256]
        bcol[0:64, 2] = bk[ks]
        bcol[64:128, 2] = bv[ks]
        woT = np.ascontiguousarray(wo[:, qs].T).astype(BF)  # [256, DIM]
        esink = np.tile(np.exp(sinks[c * QH : (c + 1) * QH]).reshape(1, QH),
                        (128, 1))
        in_maps.append(
            {
                "xT": xT,
                "wproj": np.ascontiguousarray(wproj).astype(BF),
                "bproj": bcol,
                "cos2": cos2,
                "sin2": sin2,
                "pblk": pblk,
                "woT": woT,
                "esink": esink.astype(np.float32),
                "mneg": mneg,
                "idb": ident,
                "ones_f": np.ones((128, 128), np.float32),
                "onesb": np.ones((128, 1), BF),
            }
        )
    return in_maps


def build_nc(split_waits=True):
    nc = bass.Bass("TRN2", target_bir_lowering=False, debug=False, num_devices=NCORES)
    xT = nc.dram_tensor("xT", [DIM, S], BF16, kind="ExternalInput").ap()
    wproj = nc.dram_tensor("wproj", [DIM, 384], BF16, kind="ExternalInput").ap()
    bproj = nc.dram_tensor("bproj", [128, 3], F32, kind="ExternalInput").ap()
    cos2 = nc.dram_tensor("cos2", [128, S], BF16, kind="ExternalInput").ap()
    sin2 = nc.dram_tensor("sin2", [128, S], BF16, kind="ExternalInput").ap()
    pblk = nc.dram_tensor("pblk", [128, 128], BF16, kind="ExternalInput").ap()
    woT = nc.dram_tensor("woT", [2 * 128, DIM], BF16, kind="ExternalInput").ap()
    esink = nc.dram_tensor("esink", [128, QH], F32, kind="ExternalInput").ap()
    mneg = nc.dram_tensor("mneg", [128, 128], BF16, kind="ExternalInput").ap()
    idb = nc.dram_tensor("idb", [128, 128], BF16, kind="ExternalInput").ap()
    ones_f = nc.dram_tensor("ones_f", [128, 128], F32, kind="ExternalInput").ap()
    onesb = nc.dram_tensor("onesb", [128, 1], BF16, kind="ExternalInput").ap()
    out = nc.dram_tensor("out", [S, DIM], F32, kind="ExternalOutput").ap()

    NT = S // 128  # 16 sk tiles

    with tile.TileContext(nc) as tc:
        with (
            tc.tile_pool(name="persist", bufs=1) as P,
            tc.tile_pool(name="projw", bufs=1) as PW,
            tc.tile_pool(name="tmp", bufs=3) as TMP,
            tc.tile_pool(name="ptp", bufs=4) as PTP,
            tc.tile_pool(name="rows", bufs=2) as RP,
            tc.tile_pool(name="rbp", bufs=2) as RBP,
            tc.tile_pool(name="osp", bufs=2) as OS,
            tc.tile_pool(name="oev", bufs=4) as OE,
            # PSUM budget (8 banks): proj accum 2 + scores [128,1024]=2
            # + pso 2 + mix (rot/vtrans/rb/psf) 2
            tc.tile_pool(name="pp", bufs=2, space="PSUM") as PP,
            tc.tile_pool(name="pss", bufs=1, space="PSUM") as PSS,
            tc.tile_pool(name="pso", bufs=1, space="PSUM") as PSO,
            tc.tile_pool(name="mix", bufs=2, space="PSUM") as MIX,
        ):
            esink_t = P.tile([128, QH], F32, tag="esink")
            mneg_t = P.tile([128, 128], BF16, tag="mneg")
            idb_t = P.tile([128, 128], BF16, tag="idb")
            wo_t = [P.tile([128, DIM], BF16, name=f"wo{i}", tag=f"wo{i}")
                    for i in range(2)]
            ones_ft = P.tile([128, 128], F32, tag="ones_ft")
            cos_t = P.tile([128, S], BF16, tag="cos")
            sin_t = P.tile([128, S], BF16, tag="sin")
            pblk_t = P.tile([128, 128], BF16, tag="pblk")
            onesb_t = P.tile([128, 1], BF16, tag="onesb")
            scr = P.tile([1, 16], F32, tag="scr")
            qp = [P.tile([128, S], BF16, name=f"qp{i}", tag=f"qp{i}") for i in range(2)]
            kvraw = P.tile([128, S], BF16, tag="kvraw")
            kT2 = P.tile([128, S], BF16, tag="kT2")
            vext = P.tile([128, NT * (HD + 1)], BF16, tag="vext")

            nc.gpsimd.dma_start(esink_t[:], esink[:])
            nc.gpsimd.dma_start(mneg_t[:], mneg[:])
            nc.gpsimd.dma_start(idb_t[:], idb[:])
            nc.gpsimd.dma_start(ones_ft[:], ones_f[:])
            nc.gpsimd.dma_start(cos_t[:], cos2[:])
            nc.gpsimd.dma_start(sin_t[:], sin2[:])
            nc.gpsimd.dma_start(pblk_t[:], pblk[:])
            nc.gpsimd.dma_start(onesb_t[:], onesb[:])
            # warm the natural_log_exp table set off the critical path
            nc.scalar.activation(scr[0:1, 0:3], esink_t[0:1, 0:3], AF.Exp)
            nc.scalar.activation(scr[0:1, 0:3], scr[0:1, 0:3], AF.Ln)
            # ones columns of Vext (persist; v copies never touch them)
            for t in range(NT):
                nc.vector.tensor_copy(vext[:, t * 65 + 64 : t * 65 + 65], onesb_t[:])

            w_t, x_t = [], []
            for dc in range(NDC):
                wt = PW.tile([128, 384], BF16, name=f"w{dc}", tag=f"w{dc}")
                nc.gpsimd.dma_start(wt[:], wproj[dc * 128 : (dc + 1) * 128, :])
                w_t.append(wt)
            bcol_t = PW.tile([128, 3], F32, tag="bcol")
            nc.gpsimd.dma_start(bcol_t[:], bproj[:])
            for dc in range(NDC):
                xt = PW.tile([128, S], BF16, name=f"x{dc}", tag=f"x{dc}")
                nc.sync.dma_start(
                    xt[:, 0:SBLK], xT[dc * 128 : (dc + 1) * 128, 0:SBLK]
                )
                x_t.append(xt)
            for dc in range(NDC):
                nc.sync.dma_start(
                    x_t[dc][:, SBLK:S], xT[dc * 128 : (dc + 1) * 128, SBLK:S]
                )
            for i in range(2):
                nc.gpsimd.dma_start(wo_t[i][:], woT[i * 128 : (i + 1) * 128, :])

            # ---- projection work units (interleaved into attention) ----
            # unit kinds: "mm" = free filler; "act" = ScalarE eviction
            # (stop draining after it so its dependent PE op lands in a
            # later iteration); "pedep" = PE op depending on a recent
            # eviction (stop draining after it).
            def gen_proj_units(sb):
                ss = slice(sb * SBLK, (sb + 1) * SBLK)
                units = []
                acc = {}

                def chunk(j, c0, c1, dc):
                    if dc == 0:
                        acc[j] = PP.tile([128, SBLK], F32,
                                         name=f"pp{sb}_{j}", tag="pp")
                    nc.tensor.matmul(
                        acc[j][:], w_t[dc][:, c0:c1], x_t[dc][:, ss],
                        start=(dc == 0), stop=(dc == NDC - 1),
                    )

                def evict_q(i):
                    qr = TMP.tile([128, SBLK], BF16, name="qr", tag=f"qr{i}")
                    nc.scalar.activation(
                        qr[:], acc[i][:], AF.Identity, bias=bcol_t[:, i : i + 1]
                    )
                    acc[f"qr{i}"] = qr

                def rope_q(i):
                    qr = acc[f"qr{i}"]
                    psr = MIX.tile([128, SBLK], F32, name="psr", tag="mix")
                    nc.tensor.matmul(psr[:], pblk_t[:], qr[:],
                                     start=True, stop=True)
                    t2 = TMP.tile([128, SBLK], BF16, name="t2", tag="t2")
                    nc.vector.tensor_tensor(t2[:], psr[:], sin_t[:, ss], op=OP.mult)
                    t1 = TMP.tile([128, SBLK], BF16, name="t1", tag="t1")
                    nc.vector.tensor_tensor(t1[:], qr[:], cos_t[:, ss], op=OP.mult)
                    nc.vector.tensor_tensor(qp[i][:, ss], t1[:], t2[:], op=OP.add)

                def evict_kv():
                    nc.scalar.activation(
                        kvraw[:, ss], acc[2][:], AF.Identity, bias=bcol_t[:, 2:3]
                    )

                def rope_k():
                    psrk = MIX.tile([128, SBLK], F32, name="psrk", tag="mix")
                    nc.tensor.matmul(psrk[0:64, :], pblk_t[0:64, 0:64],
                                     kvraw[0:64, ss], start=True, stop=True)
                    t2k = TMP.tile([64, SBLK], BF16, name="t2k", tag="t2k")
                    nc.vector.tensor_tensor(
                        t2k[:], psrk[0:64, :], sin_t[0:64, ss], op=OP.mult
                    )
                    t1k = TMP.tile([64, SBLK], BF16, name="t1k", tag="t1k")
                    nc.vector.tensor_tensor(
                        t1k[:], kvraw[0:64, ss], cos_t[0:64, ss], op=OP.mult
                    )
                    nc.vector.tensor_tensor(kT2[0:64, ss], t1k[:], t2k[:], op=OP.add)
                    nc.vector.tensor_copy(kT2[64:128, ss], kT2[0:64, ss])

                def vtrans(t):
                    pv = MIX.tile([128, SBLK], F32, name="pv", tag="mix")
                    pvb = pv[:].bitcast(BF16)
                    nc.tensor.matmul(
                        pvb[:, 0:HD],
                        kvraw[64:128, t * 128 : (t + 1) * 128],
                        idb_t[64:128, 64:128],
                        is_transpose=True,
                        tile_position=(64, 0),
                    )
                    nc.vector.tensor_copy(
                        vext[:, t * 65 : t * 65 + 64], pvb[:, 0:HD]
                    )

                # self-spacing order: each eviction ("act") is followed by
                # a few matmul chunks of the NEXT output before the PE op
                # that depends on it, so the in-order PE never waits.
                for dc in range(NDC):
                    units.append(("mm", lambda dc=dc: chunk(0, 0, 128, dc)))
                units.append(("act", lambda: evict_q(0)))
                for dc in range(3):
                    units.append(("mm", lambda dc=dc: chunk(1, 128, 256, dc)))
                units.append(("pedep", lambda: rope_q(0)))
                for dc in range(3, NDC):
                    units.append(("mm", lambda dc=dc: chunk(1, 128, 256, dc)))
                units.append(("act", lambda: evict_q(1)))
                for dc in range(3):
                    units.append(("mm", lambda dc=dc: chunk(2, 256, 384, dc)))
                units.append(("pedep", lambda: rope_q(1)))
                for dc in range(3, NDC):
                    units.append(("mm", lambda dc=dc: chunk(2, 256, 384, dc)))
                units.append(("act", evict_kv))
                units.append(("pedep", rope_k))
                for t in range(4 * sb, 4 * sb + 4):
                    units.append(("pedep", lambda t=t: vtrans(t)))
                return units

            def emit_renorm2(job):
                """broadcast rinv rows + apply to pso -> outstk (bf16)"""
                pso_j, rowb, osk_p = job
                for j in range(2):
                    ps_rb = MIX.tile([128, SBLK], F32, name="ps_rb", tag="mix")
                    nc.tensor.matmul(
                        ps_rb[0:64, :],
                        ones_ft[64 * j : 64 * j + 1, 0:64],
                        rowb[64 * j : 64 * j + 1, :],
                        start=True, stop=True,
                        tile_position=(64 * j, 0),
                    )
                    rb = RBP.tile([64, SBLK], F32, name="rb", tag="rb")
                    nc.vector.tensor_copy(rb[:], ps_rb[0:64, :])
                    nc.vector.tensor_tensor(
                        osk_p[64 * j : 64 * j + 64, :],
                        pso_j[j][0:64, :],
                        rb[:],
                        op=OP.mult,
                    )

            def gen_wo_units(bb, osk):
                units = []

                def wo_st(sti):
                    st = 4 * bb + sti
                    stl = slice(sti * 128, sti * 128 + 128)
                    for dbp in range(2):
                        psf = [
                            MIX.tile([128, SBLK], F32, name="psf", tag="mix")
                            for _ in range(2)
                        ]
                        for half in range(2):
                            for k in range(2):
                                db = 2 * dbp + k
                                ds = slice(db * SBLK, (db + 1) * SBLK)
                                nc.tensor.matmul(
                                    psf[k][:],
                                    osk[half][:, stl],
                                    wo_t[half][:, ds],
                                    start=(half == 0),
                                    stop=(half == 1),
                                )
                        for k in range(2):
                            db = 2 * dbp + k
                            ds = slice(db * SBLK, (db + 1) * SBLK)
                            ot = OE.tile([128, SBLK], F32, name="ot", tag="oe")
                            nc.vector.tensor_copy(ot[:], psf[k][:])
                            nc.sync.dma_start(
                                out[st * 128 : (st + 1) * 128, ds], ot[:]
                            )

                for sti in range(4):
                    units.append(("mm", lambda sti=sti: wo_st(sti)))
                return units

            # ---- bootstrap: proj(0) fully, then the attention loop ----
            for kind, fn in gen_proj_units(0):
                fn()

            unit_q = []
            renorm_job = None
            outstk_prev = None
            for b in range(NSB):
                nt = 4 * b + 4
                bs = b * SBLK
                # everything queued for this block must be in before its
                # first scores (qp/kT2/vext of block b, wo of b-2)
                for kind, fn in unit_q:
                    fn()
                unit_q = []
                if b + 1 < NSB:
                    unit_q += gen_proj_units(b + 1)
                if outstk_prev is not None:
                    # weave wo(b-1) into the stream now (its input outstk
                    # is finalized by emit_renorm2 at t==0 below); spacing
                    # the wo tiles among proj units keeps fillers flowing
                    wou = gen_wo_units(b - 1, outstk_prev)
                    nq = []
                    while unit_q or wou:
                        take = 3
                        while unit_q and take > 0:
                            nq.append(unit_q.pop(0))
                            take -= 1
                        if wou:
                            nq.append(wou.pop(0))
                    unit_q = nq
                    outstk_prev = None
                iters_left = [2 * nt]
                osk = [
                    OS.tile([128, SBLK], BF16, name=f"os{p}", tag=f"os{p}")
                    for p in range(2)
                ]
                for p in range(2):
                    pso = [
                        PSO.tile([65, SBLK], F32, name=f"oo{j}", tag=f"oo{j}")
                        for j in range(2)
                    ]

                    def emit_av(tt, ooff, pt):
                        for lane in range(2):
                            lo = ooff if lane == 0 else SBLK
                            nc.tensor.matmul(
                                pso[lane][:, ooff:SBLK],
                                vext[:, tt * 65 : (tt + 1) * 65],
                                pt[:, lo : lo + SBLK - ooff],
                                start=(tt == 0),
                                stop=(tt == nt - 1),
                            )

                    pend = []
                    for t in range(nt):
                        off = 128 * (t - 4 * b) if t >= 4 * b else 0
                        diag = t >= 4 * b
                        tc0 = slice(t * 128, (t + 1) * 128)
                        # av first so a stalled sc never blocks it
                        if len(pend) == 2:
                            emit_av(*pend.pop(0))
                        # deferred renorm part 2 must precede this pass's
                        # first av (pso ring reuse ordering)
                        if renorm_job is not None and t == 0:
                            emit_renorm2(renorm_job)
                            renorm_job = None
                        pss = PSS.tile([128, 2 * SBLK], F32, name="pss", tag="ss")
                        for lane in range(2):
                            # lane 1 packed left so [off : 2*SBLK-off] is
                            # one contiguous valid region for the exp
                            lo = off if lane == 0 else SBLK
                            nc.tensor.matmul(
                                pss[:, lo : lo + SBLK - off],
                                kT2[64 * lane : 64 * lane + 64, tc0],
                                qp[p][64 * lane : 64 * lane + 64,
                                     bs + off : bs + SBLK],
                                start=True,
                                stop=not diag,
                                tile_position=(64 * lane, 0),
                            )
                            if diag:
                                nc.tensor.matmul(
                                    pss[:, lo : lo + 128],
                                    idb_t[:],
                                    mneg_t[:],
                                    start=False,
                                    stop=True,
                                )
                        ptt = PTP.tile([128, 2 * SBLK], BF16, name="ptt", tag="pt")
                        nc.scalar.activation(
                            ptt[:, off : 2 * SBLK - off],
                            pss[:, off : 2 * SBLK - off],
                            AF.Exp,
                            scale=SCALE,
                        )
                        pend.append((t, off, ptt))
                        # drain filler units AFTER sc/exp so scores issue
                        # immediately at pass starts; pace so the queue
                        # lasts the whole block, keeping the PE fed
                        # through the late exp-bound iterations
                        cap = -(-len(unit_q) // max(iters_left[0], 1))
                        cap = min(max(cap, 2), 8)
                        iters_left[0] -= 1
                        nmm = 0
                        while unit_q and nmm < cap:
                            kind, fn = unit_q.pop(0)
                            fn()
                            nmm += 1
                            if kind == "pedep":
                                break
                    for pe_ in pend:
                        emit_av(*pe_)
                    # ---- renorm part 1 for heads (2p, 2p+1) ----
                    rowb = RP.tile([128, SBLK], F32, name="rowb", tag="rowb")
                    nc.gpsimd.memset(rowb[:], 1.0)
                    for j in range(2):
                        nc.vector.tensor_scalar_add(
                            rowb[64 * j : 64 * j + 1, :],
                            pso[j][64:65, :],
                            esink_t[64:65, 2 * p + j : 2 * p + j + 1],
                        )
                    rln = RP.tile([128, SBLK], F32, name="rln", tag="rln")
                    nc.scalar.activation(rln[:], rowb[:], AF.Ln)
                    nc.scalar.activation(rowb[:], rln[:], AF.Exp, scale=-1.0)
                    renorm_job = (pso, rowb, osk[p])
                    if p == 0 and outstk_prev is not None:
                        # weave wo tiles into the remaining units as spacers
                        wou = gen_wo_units(b - 1, outstk_prev)
                        nq = []
                        while unit_q or wou:
                            if wou:
                                nq.append(wou.pop(0))
                            take = 2
                            while unit_q and take > 0:
                                u = unit_q.pop(0)
                                nq.append(u)
                                take -= 1
                        unit_q = nq
                        outstk_prev = None
                outstk_prev = osk
            # tail
            for kind, fn in unit_q:
                fn()
            emit_renorm2(renorm_job)
            for kind, fn in gen_wo_units(NSB - 1, outstk_prev):
                fn()

    _fix_range_clears(nc)
    if split_waits:
        _split_excess_waits(nc)
    return nc


_nc_cache = [None]


def kernel(**inputs):
    in_maps = prep_inputs(inputs)
    if _nc_cache[0] is None:
        _nc_cache[0] = build_nc()
    nc = _nc_cache[0]
    res = run_bass_kernel_spmd(nc, in_maps, list(range(NCORES)))
    acc = res.results[0]["out"].astype(np.float32)
    for i in range(1, NCORES):
        acc = acc + res.results[i]["out"]
    acc = acc + np.asarray(inputs["wo_b"], np.float32).reshape(1, DIM)
    return acc.reshape(B, S, DIM)


# revision 39
# speedup vs baseline: 1.0175x; 1.0109x over previous
"""Trainium2 Bass kernel for nn_Attention_4037269258732 (GQA attention with
RoPE, causal mask, and per-head sink-logit LSE renormalization).

Problem:  B=1, S=2048, DIM=2048, H=32 q-heads, KVH=8 kv-heads, HD=64.
          out = Wo @ attn(RoPE(Wq x), RoPE(Wk x), Wv x) + bo, causal,
          with out rows scaled by sigmoid(lse - sink_h).

Sharding (8 cores, tensor-parallel over heads):
  core c owns q-heads [4c, 4c+4), kv-head c, the matching rows of
  wq/wk/wv, wo's input-dim slice [256c, 256c+256), and sinks[4c:4c+4].
  Each core computes a full-shape [S, DIM] fp32 partial of the output
  projection; the host sums the 8 partials and adds wo_b once.

Device dataflow per core (feature dims on SBUF partitions; projection
work for seq block sb+1 is interleaved into the attention tile stream
of block sb as filler "work units" so the in-order PE never idles):
  Projection (per 512-col seq block sb):
    qT[256,S], kT[64,S], vT[64,S] = W.T @ xT   (PSUM accumulate over 16
    DIM chunks; bias folded into the ScalarE Identity eviction)
    RoPE via PE: rot_half(q) = Pblk.T @ q_raw (signed permutation as
    stationary), then qp = q_raw*cos + rot*sin (3 bf16 DVE multiplies)
    v transposed into Vext = [v_nat | 1] via PE transpose
  Attention (per block b, per head-pair pass p, per 128-row sk tile t):
    scores: two K=64 matmuls row-packed via tile_position into one
    [128,1024] PSUM pair; causal mask added in PSUM by an extra
    I.T @ (-1e4*tril) accumulate on diagonal tiles
    ptt = exp(scores/8) as ONE [128,~1024] ACTIVATE
    pso[h] += Vext_t.T @ ptt   (row 64 = sum_exp denominator; emitted
    depth-2 software-pipelined: av(t-2) issues before sc(t))
    per pass: denom rows (+e^sink) -> Ln -> Exp(-x); the K=1 broadcast
    matmul + outstk = pso * rinv (bf16) is deferred into the next pass
    wo(b) woven into block b+1's unit stream:
    psf[sq,dim] = outstk0.T@wo0 + outstk1.T@wo1, evicted fp32 -> DRAM
    (wo_b added host-side, free)
"""

import numpy as np
import ml_dtypes

import bass_rust
import concourse.bass as bass
import concourse.tile as tile
from concourse import mybir
from concourse.bass_utils import run_bass_kernel_spmd

F32 = mybir.dt.float32
BF16 = mybir.dt.bfloat16
AF = mybir.ActivationFunctionType
OP = mybir.AluOpType
BF = ml_dtypes.bfloat16

B, S, DIM = 1, 2048, 2048
H, KVH, HD = 32, 8, 64
NCORES = 8
QH = H // NCORES          # 4 q heads per core
SBLK = 512                # sq block size
NSB = S // SBLK           # 4
NDC = DIM // 128          # 16 contraction chunks
SCALE = 1.0 / 8.0         # 1/sqrt(HD)
MASKNEG = -10000.0

_ws_ctr = [0]


def _fix_range_clears(nc):
    """walrus here rejects the EVENT_SEMAPHORE_RANGE_CLEAR ISA struct
    ("ISA wrong length"); replace with per-sem write-0 NoOps."""
    import re as _re
    for f in nc.m.functions:
        for blk in f.blocks:
            out, changed = [], False
            for inst in blk.instructions:
                if type(inst).__name__ == "InstISA" and inst.isa_opcode == 176:
                    m = _re.search(r"range_first=(\d+) range_last=(\d+)", inst.concise())
                    first, last = int(m.group(1)), int(m.group(2))
                    for semid in range(first, last + 1):
                        _ws_ctr[0] += 1
                        nop = mybir.InstNoOp(name=f"I-rc-{_ws_ctr[0]}", ins=[], outs=[])
                        nop.engine = inst.engine
                        nop.sync_info = bass_rust.SyncInfo(
                            on_wait=[],
                            on_update=[
                                bass_rust.SyncUpdate(
                                    sync_type="semaphore",
                                    id=semid,
                                    update_mode="sem-wr-imm",
                                    update_value=0,
                                )
                            ],
                        )
                        out.append(nop)
                    changed = True
                    continue
                out.append(inst)
            if changed:
                blk.instructions = out


def _split_excess_waits(nc, max_waits=1):
    """walrus on this image encodes at most one SyncWait per instruction;
    hoist excess waits onto same-engine NoOps placed just before."""
    for f in nc.m.functions:
        for blk in f.blocks:
            out, changed = [], False
            for inst in blk.instructions:
                si = inst.sync_info
                waits = list(si.on_wait) if si is not None else []
                if len(waits) > max_waits:
                    excess, keep = waits[:-max_waits], waits[-max_waits:]
                    for k in range(0, len(excess), max_waits):
                        _ws_ctr[0] += 1
                        nop = mybir.InstNoOp(name=f"I-ws-{_ws_ctr[0]}", ins=[], outs=[])
                        nop.engine = inst.engine
                        nop.sync_info = bass_rust.SyncInfo(
                            on_wait=excess[k : k + max_waits], on_update=[]
                        )
                        out.append(nop)
                    inst.sync_info = bass_rust.SyncInfo(
                        on_wait=keep, on_update=list(si.on_update)
                    )
                    changed = True
                out.append(inst)
            if changed:
                blk.instructions = out


def _rot_perm(nheads):
    """Signed permutation P with (P.T @ q)[d] = rot_half(q)[d] per head."""
    n = nheads * HD
    P = np.zeros((n, n), np.float32)
    for d in range(n):
        j, dh = d // HD, d % HD
        src = j * HD + (dh + 32) % HD
        P[src, d] = -1.0 if dh < 32 else 1.0
    return P


def prep_inputs(inputs):
    """Host-side sharding/layout prep. Returns per-core input maps."""
    x = np.asarray(inputs["x"], np.float32)
    rope = np.asarray(inputs["rope_cache"], np.float32)
    wq = np.asarray(inputs["wq_w"], np.float32)
    bq = np.asarray(inputs["wq_b"], np.float32)
    wk = np.asarray(inputs["wk_w"], np.float32)
    bk = np.asarray(inputs["wk_b"], np.float32)
    wv = np.asarray(inputs["wv_w"], np.float32)
    bv = np.asarray(inputs["wv_b"], np.float32)
    wo = np.asarray(inputs["wo_w"], np.float32)
    sinks = np.asarray(inputs["sinks"], np.float32)

    xT = np.ascontiguousarray(x[0].T).astype(BF)            # [DIM, S]
    cosT = rope[:, :HD].T                                   # [64, S]
    sinT = rope[:, HD:].T
    cos2 = np.ascontiguousarray(np.concatenate([cosT, cosT], 0)).astype(BF)
    sin2 = np.ascontiguousarray(np.concatenate([sinT, sinT], 0)).astype(BF)
    pblk = np.ascontiguousarray(_rot_perm(2)).astype(BF)    # [128,128]
    mneg = (MASKNEG * np.tril(np.ones((128, 128), np.float32), -1)).astype(BF)
    ident = np.eye(128, dtype=BF)

    in_maps = []
    for c in range(NCORES):
        qs = slice(c * QH * HD, (c + 1) * QH * HD)          # 256 q rows
        ks = slice(c * HD, (c + 1) * HD)                    # 64 kv rows
        # wproj columns: [q 256 | k 64 | v 64] = 384
        wproj = np.concatenate([wq[qs].T, wk[ks].T, wv[ks].T], axis=1)
        bcol = np.zeros((128, 3), np.float32)
        bcol[:, 0] = bq[qs][0:128]
        bcol[:, 1] = bq[qs][128:256]
        bcol[0:64, 2] = bk[ks]
        bcol[64:128, 2] = bv[ks]
        woT = np.ascontiguousarray(wo[:, qs].T).astype(BF)  # [256, DIM]
        esink = np.tile(np.exp(sinks[c * QH : (c + 1) * QH]).reshape(1, QH),
                        (128, 1))
        in_maps.append(
            {
                "xT": xT,
                "wproj": np.ascontiguousarray(wproj).astype(BF),
                "bproj": bcol,
                "cos2": cos2,
                "sin2": sin2,
                "pblk": pblk,
                "woT": woT,
                "esink": esink.astype(np.float32),
                "mneg": mneg,
                "idb": ident,
                "ones_f": np.ones((128, 128), np.float32),
                "onesb": np.ones((128, 1), BF),
            }
        )
    return in_maps


def build_nc(split_waits=True):
    nc = bass.Bass("TRN2", target_bir_lowering=False, debug=False, num_devices=NCORES)
    xT = nc.dram_tensor("xT", [DIM, S], BF16, kind="ExternalInput").ap()
    wproj = nc.dram_tensor("wproj", [DIM, 384], BF16, kind="ExternalInput").ap()
    bproj = nc.dram_tensor("bproj", [128, 3], F32, kind="ExternalInput").ap()
    cos2 = nc.dram_tensor("cos2", [128, S], BF16, kind="ExternalInput").ap()
    sin2 = nc.dram_tensor("sin2", [128, S], BF16, kind="ExternalInput").ap()
    pblk = nc.dram_tensor("pblk", [128, 128], BF16, kind="ExternalInput").ap()
    woT = nc.dram_tensor("woT", [2 * 128, DIM], BF16, kind="ExternalInput").ap()
    esink = nc.dram_tensor("esink", [128, QH], F32, kind="ExternalInput").ap()
    mneg = nc.dram_tensor("mneg", [128, 128], BF16, kind="ExternalInput").ap()
    idb = nc.dram_tensor("idb", [128, 128], BF16, kind="ExternalInput").ap()
    ones_f = nc.dram_tensor("ones_f", [128, 128], F32, kind="ExternalInput").ap()
    onesb = nc.dram_tensor("onesb", [128, 1], BF16, kind="ExternalInput").ap()
    out = nc.dram_tensor("out", [S, DIM], F32, kind="ExternalOutput").ap()

    NT = S // 128  # 16 sk tiles

    with tile.TileContext(nc) as tc:
        with (
            tc.tile_pool(name="persist", bufs=1) as P,
            tc.tile_pool(name="projw", bufs=1) as PW,
            tc.tile_pool(name="tmp", bufs=3) as TMP,
            tc.tile_pool(name="ptp", bufs=4) as PTP,
            tc.tile_pool(name="rows", bufs=2) as RP,
            tc.tile_pool(name="rbp", bufs=2) as RBP,
            tc.tile_pool(name="osp", bufs=2) as OS,
            tc.tile_pool(name="oev", bufs=4) as OE,
            # PSUM budget (8 banks): proj accum 2 + scores [128,1024]=2
            # + pso 2 + mix (rot/vtrans/rb/psf) 2
            tc.tile_pool(name="pp", bufs=2, space="PSUM") as PP,
            tc.tile_pool(name="pss", bufs=1, space="PSUM") as PSS,
            tc.tile_pool(name="pso", bufs=1, space="PSUM") as PSO,
            tc.tile_pool(name="mix", bufs=2, space="PSUM") as MIX,
        ):
            esink_t = P.tile([128, QH], F32, tag="esink")
            mneg_t = P.tile([128, 128], BF16, tag="mneg")
            idb_t = P.tile([128, 128], BF16, tag="idb")
            wo_t = [P.tile([128, DIM], BF16, name=f"wo{i}", tag=f"wo{i}")
                    for i in range(2)]
            ones_ft = P.tile([128, 128], F32, tag="ones_ft")
            cos_t = P.tile([128, S], BF16, tag="cos")
            sin_t = P.tile([128, S], BF16, tag="sin")
            pblk_t = P.tile([128, 128], BF16, tag="pblk")
            onesb_t = P.tile([128, 1], BF16, tag="onesb")
            scr = P.tile([1, 16], F32, tag="scr")
            qp = [P.tile([128, S], BF16, name=f"qp{i}", tag=f"qp{i}") for i in range(2)]
            kvraw = P.tile([128, S], BF16, tag="kvraw")
            kT2 = P.tile([128, S], BF16, tag="kT2")
            vext = P.tile([128, NT * (HD + 1)], BF16, tag="vext")

            nc.gpsimd.dma_start(esink_t[:], esink[:])
            nc.gpsimd.dma_start(mneg_t[:], mneg[:])
            nc.gpsimd.dma_start(idb_t[:], idb[:])
            nc.gpsimd.dma_start(ones_ft[:], ones_f[:])
            nc.gpsimd.dma_start(cos_t[:], cos2[:])
            nc.gpsimd.dma_start(sin_t[:], sin2[:])
            nc.gpsimd.dma_start(pblk_t[:], pblk[:])
            nc.gpsimd.dma_start(onesb_t[:], onesb[:])
            # warm the natural_log_exp table set off the critical path
            nc.scalar.activation(scr[0:1, 0:3], esink_t[0:1, 0:3], AF.Exp)
            nc.scalar.activation(scr[0:1, 0:3], scr[0:1, 0:3], AF.Ln)
            # ones columns of Vext (persist; v copies never touch them)
            for t in range(NT):
                nc.vector.tensor_copy(vext[:, t * 65 + 64 : t * 65 + 65], onesb_t[:])

            w_t, x_t = [], []
            for dc in range(NDC):
                wt = PW.tile([128, 384], BF16, name=f"w{dc}", tag=f"w{dc}")
                nc.gpsimd.dma_start(wt[:], wproj[dc * 128 : (dc + 1) * 128, :])
                w_t.append(wt)
            bcol_t = PW.tile([128, 3], F32, tag="bcol")
            nc.gpsimd.dma_start(bcol_t[:], bproj[:])
            for dc in range(NDC):
                xt = PW.tile([128, S], BF16, name=f"x{dc}", tag=f"x{dc}")
                nc.sync.dma_start(
                    xt[:, 0:SBLK], xT[dc * 128 : (dc + 1) * 128, 0:SBLK]
                )
                x_t.append(xt)
            for dc in range(NDC):
                nc.sync.dma_start(
                    x_t[dc][:, SBLK:S], xT[dc * 128 : (dc + 1) * 128, SBLK:S]
                )
            for i in range(2):
                nc.gpsimd.dma_start(wo_t[i][:], woT[i * 128 : (i + 1) * 128, :])

            # ---- projection work units (interleaved into attention) ----
            # unit kinds: "mm" = free filler; "act" = ScalarE eviction
            # (stop draining after it so its dependent PE op lands in a
            # later iteration); "pedep" = PE op depending on a recent
            # eviction (stop draining after it).
            def gen_proj_units(sb):
                ss = slice(sb * SBLK, (sb + 1) * SBLK)
                units = []
                acc = {}

                def chunk(j, c0, c1, dc):
                    if dc == 0:
                        acc[j] = PP.tile([128, SBLK], F32,
                                         name=f"pp{sb}_{j}", tag="pp")
                    nc.tensor.matmul(
                        acc[j][:], w_t[dc][:, c0:c1], x_t[dc][:, ss],
                        start=(dc == 0), stop=(dc == NDC - 1),
                    )

                def evict_q(i):
                    qr = TMP.tile([128, SBLK], BF16, name="qr", tag=f"qr{i}")
                    nc.scalar.activation(
                        qr[:], acc[i][:], AF.Identity, bias=bcol_t[:, i : i + 1]
                    )
                    acc[f"qr{i}"] = qr

                def rope_q(i):
                    qr = acc[f"qr{i}"]
                    psr = MIX.tile([128, SBLK], F32, name="psr", tag="mix")
                    nc.tensor.matmul(psr[:], pblk_t[:], qr[:],
                                     start=True, stop=True)
                    t2 = TMP.tile([128, SBLK], BF16, name="t2", tag="t2")
                    nc.vector.tensor_tensor(t2[:], psr[:], sin_t[:, ss], op=OP.mult)
                    t1 = TMP.tile([128, SBLK], BF16, name="t1", tag="t1")
                    nc.vector.tensor_tensor(t1[:], qr[:], cos_t[:, ss], op=OP.mult)
                    nc.vector.tensor_tensor(qp[i][:, ss], t1[:], t2[:], op=OP.add)

                def evict_kv():
                    nc.scalar.activation(
                        kvraw[:, ss], acc[2][:], AF.Identity, bias=bcol_t[:, 2:3]
                    )

                def rope_k():
                    psrk = MIX.tile([128, SBLK], F32, name="psrk", tag="mix")
                    nc.tensor.matmul(psrk[0:64, :], pblk_t[0:64, 0:64],
                                     kvraw[0:64, ss], start=True, stop=True)
                    t2k = TMP.tile([64, SBLK], BF16, name="t2k", tag="t2k")
                    nc.vector.tensor_tensor(
                        t2k[:], psrk[0:64, :], sin_t[0:64, ss], op=OP.mult
                    )
                    t1k = TMP.tile([64, SBLK], BF16, name="t1k", tag="t1k")
                    nc.vector.tensor_tensor(
                        t1k[:], kvraw[0:64, ss], cos_t[0:64, ss], op=OP.mult
                    )
                    nc.vector.tensor_tensor(kT2[0:64, ss], t1k[:], t2k[:], op=OP.add)
                    nc.vector.tensor_copy(kT2[64:128, ss], kT2[0:64, ss])

                def vtrans(t):
                    pv = MIX.tile([128, SBLK], F32, name="pv", tag="mix")
                    pvb = pv[:].bitcast(BF16)
                    nc.tensor.matmul(
                        pvb[:, 0:HD],
                        kvraw[64:128, t * 128 : (t + 1) * 128],
                        idb_t[64:128, 64:128],
                        is_transpose=True,
                        tile_position=(64, 0),
                    )
                    nc.vector.tensor_copy(
                        vext[:, t * 65 : t * 65 + 64], pvb[:, 0:HD]
                    )

                # self-spacing order: each eviction ("act") is followed by
                # matmul chunks of the NEXT output before the PE ops that
                # depend on it, so the in-order PE never waits. kv is
                # produced second so q1's chunks space out rope_k and the
                # four v-transposes; rope_q1 (needed only by pass 1 of the
                # next block) trails.
                for dc in range(NDC):
                    units.append(("mm", lambda dc=dc: chunk(0, 0, 128, dc)))
                units.append(("act", lambda: evict_q(0)))
                for dc in range(3):
                    units.append(("mm", lambda dc=dc: chunk(2, 256, 384, dc)))
                units.append(("pedep", lambda: rope_q(0)))
                for dc in range(3, NDC):
                    units.append(("mm", lambda dc=dc: chunk(2, 256, 384, dc)))
                units.append(("act", evict_kv))
                for dc in range(3):
                    units.append(("mm", lambda dc=dc: chunk(1, 128, 256, dc)))
                units.append(("pedep", rope_k))
                for ti, t in enumerate(range(4 * sb, 4 * sb + 4)):
                    for dc in range(3 + 3 * ti, 6 + 3 * ti):
                        units.append(("mm", lambda dc=dc: chunk(1, 128, 256, dc)))
                    units.append(("pedep", lambda t=t: vtrans(t)))
                units.append(("mm", lambda: chunk(1, 128, 256, 15)))
                units.append(("act", lambda: evict_q(1)))
                units.append(("pedep", lambda: rope_q(1)))
                return units

            def emit_renorm2(job):
                """broadcast rinv rows + apply to pso -> outstk (bf16)"""
                pso_j, rowb, osk_p = job
                for j in range(2):
                    ps_rb = MIX.tile([128, SBLK], F32, name="ps_rb", tag="mix")
                    nc.tensor.matmul(
                        ps_rb[0:64, :],
                        ones_ft[64 * j : 64 * j + 1, 0:64],
                        rowb[64 * j : 64 * j + 1, :],
                        start=True, stop=True,
                        tile_position=(64 * j, 0),
                    )
                    rb = RBP.tile([64, SBLK], F32, name="rb", tag="rb")
                    nc.vector.tensor_copy(rb[:], ps_rb[0:64, :])
                    nc.vector.tensor_tensor(
                        osk_p[64 * j : 64 * j + 64, :],
                        pso_j[j][0:64, :],
                        rb[:],
                        op=OP.mult,
                    )

            def gen_wo_units(bb, osk):
                units = []

                def wo_st(sti):
                    st = 4 * bb + sti
                    stl = slice(sti * 128, sti * 128 + 128)
                    for dbp in range(2):
                        psf = [
                            MIX.tile([128, SBLK], F32, name="psf", tag="mix")
                            for _ in range(2)
                        ]
                        for half in range(2):
                            for k in range(2):
                                db = 2 * dbp + k
                                ds = slice(db * SBLK, (db + 1) * SBLK)
                                nc.tensor.matmul(
                                    psf[k][:],
                                    osk[half][:, stl],
                                    wo_t[half][:, ds],
                                    start=(half == 0),
                                    stop=(half == 1),
                                )
                        for k in range(2):
                            db = 2 * dbp + k
                            ds = slice(db * SBLK, (db + 1) * SBLK)
                            ot = OE.tile([128, SBLK], F32, name="ot", tag="oe")
                            nc.vector.tensor_copy(ot[:], psf[k][:])
                            nc.sync.dma_start(
                                out[st * 128 : (st + 1) * 128, ds], ot[:]
                            )

                for sti in range(4):
                    units.append(("mm", lambda sti=sti: wo_st(sti)))
                return units

            # ---- bootstrap: proj(0) fully, then the attention loop ----
            for kind, fn in gen_proj_units(0):
                fn()

            unit_q = []
            renorm_job = None
            outstk_prev = None
            for b in range(NSB):
                nt = 4 * b + 4
                bs = b * SBLK
                # everything queued for this block must be in before its
                # first scores (qp/kT2/vext of block b, wo of b-2)
                for kind, fn in unit_q:
                    fn()
                unit_q = []
                if b + 1 < NSB:
                    unit_q += gen_proj_units(b + 1)
                if outstk_prev is not None:
                    # weave wo(b-1) into the stream now (its input outstk
                    # is finalized by emit_renorm2 at t==0 below); spacing
                    # the wo tiles among proj units keeps fillers flowing
                    wou = gen_wo_units(b - 1, outstk_prev)
                    nq = []
                    while unit_q or wou:
                        take = 3
                        while unit_q and take > 0:
                            nq.append(unit_q.pop(0))
                            take -= 1
                        if wou:
                            nq.append(wou.pop(0))
                    unit_q = nq
                    outstk_prev = None
                iters_left = [2 * nt]
                osk = [
                    OS.tile([128, SBLK], BF16, name=f"os{p}", tag=f"os{p}")
                    for p in range(2)
                ]
                for p in range(2):
                    pso = [
                        PSO.tile([65, SBLK], F32, name=f"oo{j}", tag=f"oo{j}")
                        for j in range(2)
                    ]

                    def emit_av(tt, ooff, pt):
                        for lane in range(2):
                            lo = ooff if lane == 0 else SBLK
                            nc.tensor.matmul(
                                pso[lane][:, ooff:SBLK],
                                vext[:, tt * 65 : (tt + 1) * 65],
                                pt[:, lo : lo + SBLK - ooff],
                                start=(tt == 0),
                                stop=(tt == nt - 1),
                            )

                    pend = []
                    for t in range(nt):
                        off = 128 * (t - 4 * b) if t >= 4 * b else 0
                        diag = t >= 4 * b
                        tc0 = slice(t * 128, (t + 1) * 128)
                        # av first so a stalled sc never blocks it
                        if len(pend) == 2:
                            emit_av(*pend.pop(0))
                        # deferred renorm part 2 must precede this pass's
                        # first av (pso ring reuse ordering)
                        if renorm_job is not None and t == 0:
                            emit_renorm2(renorm_job)
                            renorm_job = None
                        pss = PSS.tile([128, 2 * SBLK], F32, name="pss", tag="ss")
                        for lane in range(2):
                            # lane 1 packed left so [off : 2*SBLK-off] is
                            # one contiguous valid region for the exp
                            lo = off if lane == 0 else SBLK
                            nc.tensor.matmul(
                                pss[:, lo : lo + SBLK - off],
                                kT2[64 * lane : 64 * lane + 64, tc0],
                                qp[p][64 * lane : 64 * lane + 64,
                                     bs + off : bs + SBLK],
                                start=True,
                                stop=not diag,
                                tile_position=(64 * lane, 0),
                            )
                            if diag:
                                nc.tensor.matmul(
                                    pss[:, lo : lo + 128],
                                    idb_t[:],
                                    mneg_t[:],
                                    start=False,
                                    stop=True,
                                )
                        ptt = PTP.tile([128, 2 * SBLK], BF16, name="ptt", tag="pt")
                        nc.scalar.activation(
                            ptt[:, off : 2 * SBLK - off],
                            pss[:, off : 2 * SBLK - off],
                            AF.Exp,
                            scale=SCALE,
                        )
                        pend.append((t, off, ptt))
                        # drain filler units AFTER sc/exp so scores issue
                        # immediately at pass starts; pace so the queue
                        # lasts the whole block, keeping the PE fed
                        # through the late exp-bound iterations
                        cap = -(-len(unit_q) // max(iters_left[0], 1))
                        cap = min(max(cap, 2), 8)
                        iters_left[0] -= 1
                        nmm = 0
                        while unit_q and nmm < cap:
                            kind, fn = unit_q.pop(0)
                            fn()
                            nmm += 1
                            if kind == "pedep":
                                break
                    for pe_ in pend:
                        emit_av(*pe_)
                    # ---- renorm part 1 for heads (2p, 2p+1) ----
                    rowb = RP.tile([128, SBLK], F32, name="rowb", tag="rowb")
                    nc.gpsimd.memset(rowb[:], 1.0)
                    for j in range(2):
                        nc.vector.tensor_scalar_add(
                            rowb[64 * j : 64 * j + 1, :],
                            pso[j][64:65, :],
                            esink_t[64:65, 2 * p + j : 2 * p + j + 1],
                        )
                    rln = RP.tile([128, SBLK], F32, name="rln", tag="rln")
                    nc.scalar.activation(rln[:], rowb[:], AF.Ln)
                    nc.scalar.activation(rowb[:], rln[:], AF.Exp, scale=-1.0)
                    renorm_job = (pso, rowb, osk[p])
                    if p == 0 and outstk_prev is not None:
                        # weave wo tiles into the remaining units as spacers
                        wou = gen_wo_units(b - 1, outstk_prev)
                        nq = []
                        while unit_q or wou:
                            if wou:
                                nq.append(wou.pop(0))
                            take = 2
                            while unit_q and take > 0:
                                u = unit_q.pop(0)
                                nq.append(u)
                                take -= 1
                        unit_q = nq
                        outstk_prev = None
                outstk_prev = osk
            # tail
            for kind, fn in unit_q:
                fn()
            emit_renorm2(renorm_job)
            for kind, fn in gen_wo_units(NSB - 1, outstk_prev):
                fn()

    _fix_range_clears(nc)
    if split_waits:
        _split_excess_waits(nc)
    return nc


_nc_cache = [None]


def kernel(**inputs):
    in_maps = prep_inputs(inputs)
    if _nc_cache[0] is None:
        _nc_cache[0] = build_nc()
    nc = _nc_cache[0]
    res = run_bass_kernel_spmd(nc, in_maps, list(range(NCORES)))
    acc = res.results[0]["out"].astype(np.float32)
    for i in range(1, NCORES):
        acc = acc + res.results[i]["out"]
    acc = acc + np.asarray(inputs["wo_b"], np.float32).reshape(1, DIM)
    return acc.reshape(B, S, DIM)
